# revision 40
# baseline (speedup 1.0000x reference)
"""nn_GBEncoderBlock on 8 Trainium2 NeuronCores.

Sharding: data-parallel over batch (B=8 -> 1 element/core, no collectives).

On-chip layout is channel-major xT [H=512, L=1024] (4 tiles of 128
partitions). Layernorms reduce over H (= partitions) via ones-vector
matmuls on the PE; mean/rstd rows are broadcast back to [128, L] with K=1
matmuls and cached as bf16 SBUF tiles so every elementwise consumer runs
in the DVE 2x (16-bit packed) mode. Gamma is folded into the following
matmul weights host-side; beta/bias terms become per-partition biases of
the PSUM-evicting activation ops.

Matmuls run in bf16 (fp32 PSUM accumulate). The depthwise conv runs as 7
scaled-shift taps spread across DVE (4x tensor_scalar + 2x tensor_tensor
pairs), GpSimd and ACT (scale-copies), with the LN shift term folded
through depthwise+pointwise into one K=14 matmul over shifted copies of
the mean row. Softmax: scores are computed transposed ([key, query]) so
the key mask is a per-partition bias of the fused exp activation; the
denominator comes free as a 65th row of the attn@V matmul via an appended
ones column in the stationary operand. The head loop is software
pipelined (scores/exp of head h issue ahead of attnV of head h-1).
"""

import contextlib
import numpy as np
import ml_dtypes

B, L, H = 8, 1024, 512
NHEAD, DK = 8, 64
KSZ, NLAYERS = 7, 4
F = 4 * H
EPS = 1e-6
P = 128
HT = H // P      # 4  h-chunks
LC = L // 512    # 2  l-chunks of 512
MC = L // P      # 8  key chunks of 128
FT = F // P      # 16 ffn chunks
NEG = -10000.0
LN_N = float(H)
PAD = KSZ // 2   # 3
LPAD = L + KSZ - 1
BF16 = ml_dtypes.bfloat16

_CACHE = {}


def _nc_build(stop_stage="full"):
    import concourse.bass as bass  # noqa: F401
    from concourse import bacc, mybir
    import concourse.tile as tile

    dt = mybir.dt
    MM_DT = dt.bfloat16
    AL = mybir.AluOpType
    AF = mybir.ActivationFunctionType

    nc = bacc.Bacc(None, target_bir_lowering=False, debug=False)

    # ---------------- DRAM I/O ----------------
    d_xT = nc.dram_tensor("xT", [H, L], MM_DT, kind="ExternalInput")
    d_posT = nc.dram_tensor("posT", [H, L], MM_DT, kind="ExternalInput")
    d_maskb = nc.dram_tensor("maskb", [P, MC], dt.float32, kind="ExternalInput")
    d_pwT = nc.dram_tensor("pwT", [NLAYERS, H, H], MM_DT, kind="ExternalInput")
    d_uconv = nc.dram_tensor("uconv", [2 * KSZ, NLAYERS, H], MM_DT, kind="ExternalInput")
    d_dww = nc.dram_tensor("dww", [P, NLAYERS, HT, KSZ], dt.float32, kind="ExternalInput")
    d_pwb = nc.dram_tensor("pwb", [P, NLAYERS, HT], dt.float32, kind="ExternalInput")
    d_wqk = nc.dram_tensor("wqk", [H, NHEAD * 2 * DK], MM_DT, kind="ExternalInput")
    d_qkb = nc.dram_tensor("qkb", [P, NHEAD], dt.float32, kind="ExternalInput")
    d_wv = nc.dram_tensor("wv", [H, H], MM_DT, kind="ExternalInput")
    d_vbias = nc.dram_tensor("vbias", [1, H], MM_DT, kind="ExternalInput")
    d_projT = nc.dram_tensor("projT", [H, H], MM_DT, kind="ExternalInput")
    d_projb = nc.dram_tensor("projb", [P, HT], dt.float32, kind="ExternalInput")
    d_w1T = nc.dram_tensor("w1T", [H, F], MM_DT, kind="ExternalInput")
    d_f1b = nc.dram_tensor("f1b", [P, FT], dt.float32, kind="ExternalInput")
    d_w2T = nc.dram_tensor("w2T", [F, H], MM_DT, kind="ExternalInput")
    d_b2 = nc.dram_tensor("b2", [P, HT], dt.float32, kind="ExternalInput")
    d_out = nc.dram_tensor("out", [H, L], dt.float32, kind="ExternalOutput")

    with tile.TileContext(nc) as tc, contextlib.ExitStack() as ctx:
        singles = ctx.enter_context(tc.tile_pool(name="singles", bufs=1))
        stats = ctx.enter_context(tc.tile_pool(name="stats", bufs=2))
        work = ctx.enter_context(tc.tile_pool(name="work", bufs=3))
        resid = ctx.enter_context(tc.tile_pool(name="resid", bufs=1))
        sqp = ctx.enter_context(tc.tile_pool(name="sqp", bufs=1))
        ps_bc = ctx.enter_context(tc.tile_pool(name="ps_bc", bufs=1, space="PSUM"))

        # ---- constants ----
        ones_col = singles.tile([P, 1], MM_DT)            # K=128 -> M=1 column sums
        nc.vector.memset(ones_col, 1.0)
        ones_row_f32 = singles.tile([1, P], dt.float32)    # K=1 -> M=128 broadcast
        nc.vector.memset(ones_row_f32, 1.0)
        ones_row_bf = singles.tile([1, P], MM_DT)
        nc.vector.memset(ones_row_bf, 1.0)
        ones_row_dk = singles.tile([1, DK], MM_DT)         # K=1 -> M=64 broadcast
        nc.vector.memset(ones_row_dk, 1.0)
        maskb = singles.tile([P, MC], dt.float32)
        nc.sync.dma_start(out=maskb, in_=d_maskb[:, :])

        # ---- residual stream xc = xT + posT ----
        xc = []
        for hc in range(HT):
            xt = work.tile([P, L], MM_DT, tag="ld_x")
            pt = work.tile([P, L], MM_DT, tag="ld_pos")
            nc.sync.dma_start(out=xt, in_=d_xT[hc * P:(hc + 1) * P, :])
            nc.sync.dma_start(out=pt, in_=d_posT[hc * P:(hc + 1) * P, :])
            xr = resid.tile([P, L], MM_DT, tag=f"xc{hc}")
            nc.vector.tensor_tensor(xr, xt, pt, AL.add)
            xc.append(xr)

        def ln_stats(ps, stat_bufs=2, need_mean=False, mm_out=None):
            """LN over channels.

            Returns (rstd_sb [128,L] bf16, m_sb [128,L] bf16 | None).
            If mm_out is given (a [1, LPAD] padded row), writes -mean*rstd
            into its [PAD:PAD+L] span.
            """
            sq = []
            for hc in range(HT):
                s = sqp.tile([P, L], MM_DT, tag=f"sq{hc}")
                if hc < 2:
                    nc.vector.tensor_tensor(s, xc[hc], xc[hc], AL.mult)
                else:
                    nc.gpsimd.tensor_tensor(s, xc[hc], xc[hc], AL.mult)
                sq.append(s)
            rstd_f32 = stats.tile([1, L], dt.float32, tag="rstdf")
            rstd_sb = stats.tile([P, L], MM_DT, tag="rstd_sb", name="rstd_sb")
            m_sb = None
            if need_mean:
                m_sb = stats.tile([P, L], MM_DT, tag="m_sb", name="m_sb")
            for lc in range(LC):
                sl = slice(lc * 512, (lc + 1) * 512)
                p_sum = ps.tile([1, 512], dt.float32, tag="pstat",
                                bufs=stat_bufs, name=f"psum_{lc}")
                p_sq = ps.tile([1, 512], dt.float32, tag="pstat",
                               bufs=stat_bufs, name=f"psq_{lc}")
                for hc in range(HT):
                    nc.tensor.matmul(p_sum, ones_col, xc[hc][:, sl],
                                     start=(hc == 0), stop=(hc == HT - 1))
                for hc in range(HT):
                    nc.tensor.matmul(p_sq, ones_col, sq[hc][:, sl],
                                     start=(hc == 0), stop=(hc == HT - 1))
                sum_sb = stats.tile([1, 512], dt.float32, tag="sum_sb")
                nc.scalar.copy(sum_sb, p_sum)
                s2 = stats.tile([1, 512], dt.float32, tag="s2")
                nc.vector.tensor_tensor(s2, sum_sb, sum_sb, AL.mult)
                v1 = stats.tile([1, 512], dt.float32, tag="v1")
                nc.vector.tensor_scalar(v1, p_sq, 1.0 / (LN_N - 1.0), None, AL.mult)
                var = stats.tile([1, 512], dt.float32, tag="var")
                nc.vector.scalar_tensor_tensor(var, s2, -1.0 / (LN_N * (LN_N - 1.0)),
                                               v1, AL.mult, AL.add)
                std = stats.tile([1, 512], dt.float32, tag="std")
                nc.scalar.activation(std, var, AF.Sqrt)
                # eps dropped: std ~ O(1) here, 1e-6 is far below bf16 noise
                nc.vector.reciprocal(rstd_f32[:, sl], std)
                pb = ps_bc.tile([P, 512], dt.float32, tag="bc")
                nc.tensor.matmul(pb, ones_row_f32, rstd_f32[:, sl],
                                 start=True, stop=True)
                nc.scalar.copy(rstd_sb[:, sl], pb)
                if need_mean:
                    mrow = stats.tile([1, 512], dt.float32, tag="mrow")
                    nc.vector.tensor_scalar(mrow, sum_sb, 1.0 / LN_N, None, AL.mult)
                    pm2 = ps_bc.tile([P, 512], dt.float32, tag="bc")
                    nc.tensor.matmul(pm2, ones_row_f32, mrow, start=True, stop=True)
                    nc.scalar.copy(m_sb[:, sl], pm2)
                if mm_out is not None:
                    nc.vector.scalar_tensor_tensor(
                        mm_out[0:1, PAD + lc * 512:PAD + (lc + 1) * 512], sum_sb,
                        -1.0 / LN_N, rstd_f32[:, sl], AL.mult, AL.mult)
            return rstd_sb, m_sb

        def write_out(src_tiles):
            for hc in range(HT):
                ot = work.tile([P, L], dt.float32, tag="outt")
                nc.vector.tensor_copy(ot, src_tiles[hc])
                nc.sync.dma_start(out=d_out[hc * P:(hc + 1) * P, :], in_=ot)

        # =================== CNN layers ===================
        # LN shift term (-mean*rstd, beta) is folded through depthwise +
        # pointwise into a K=14 matmul over shifted copies of the mean row
        # (exact, incl. conv edge truncation); depthwise bias is folded into
        # the pointwise bias host-side. So hpad holds only (xc*gamma)*rstd.
        with tc.tile_pool(name="cnnw", bufs=2) as cnnw, \
             tc.tile_pool(name="cnnd", bufs=1) as cnnd, \
             tc.tile_pool(name="cnnm", bufs=2) as cnnm, \
             tc.tile_pool(name="ps_cnn", bufs=1, space="PSUM") as ps_cnn:
            uconv_sb = singles.tile([2 * KSZ, NLAYERS, H], MM_DT)
            nc.sync.dma_start(out=uconv_sb, in_=d_uconv[:, :, :])
            dww_sb = singles.tile([P, NLAYERS, HT, KSZ], dt.float32)
            nc.sync.dma_start(out=dww_sb, in_=d_dww[:, :, :, :])
            pwb_sb = singles.tile([P, NLAYERS, HT], dt.float32)
            nc.sync.dma_start(out=pwb_sb, in_=d_pwb[:, :, :])
            ones_pad = singles.tile([1, LPAD], MM_DT)
            nc.vector.memset(ones_pad[0:1, :], 0.0)
            nc.vector.memset(ones_pad[0:1, PAD:PAD + L], 1.0)

            for i in range(NLAYERS):
                pwT_sb = []
                for hc in range(HT):
                    t = cnnw.tile([P, H], MM_DT, tag=f"pwT{hc}")
                    nc.sync.dma_start(out=t, in_=d_pwT[i, hc * P:(hc + 1) * P, :])
                    pwT_sb.append(t)
                mm_pad = cnnm.tile([1, LPAD], MM_DT, tag="mm_pad",
                                   name=f"mm_pad{i}")
                nc.vector.memset(mm_pad[0:1, 0:PAD], 0.0)
                nc.vector.memset(mm_pad[0:1, LPAD - PAD:LPAD], 0.0)
                rstd_sb, _ = ln_stats(ps_cnn, mm_out=mm_pad)
                # shifted-window gather [14, L]: rows 0-6 = mm shifts,
                # rows 7-13 = ones-pad shifts (edge-exact beta term)
                mshift = cnnm.tile([2 * KSZ, L], MM_DT, tag="mshift",
                                   name=f"mshift{i}")
                mm_win = bass.AP(tensor=mm_pad.tensor, offset=mm_pad.offset,
                                 ap=[[1, 1], [1, KSZ], [1, L]])
                nc.sync.dma_start(out=mshift[0:KSZ, :], in_=mm_win)
                ones_win = bass.AP(tensor=ones_pad.tensor, offset=ones_pad.offset,
                                   ap=[[1, 1], [1, KSZ], [1, L]])
                nc.sync.dma_start(out=mshift[KSZ:2 * KSZ, :], in_=ones_win)
                # per-hc pipeline: hpad_hc -> GpSimd B-chain while DVE runs
                # the A-chain; combine right after so dw_hc frees early for
                # the kc-outer pointwise matmuls.
                def emit_hpad(hc):
                    hpad = cnnd.tile([P, LPAD], MM_DT, tag=f"hpad{hc}",
                                     name=f"hpad{i}_{hc}")
                    nc.vector.memset(hpad[:, 0:PAD], 0.0)
                    nc.vector.memset(hpad[:, LPAD - PAD:LPAD], 0.0)
                    nc.vector.tensor_tensor(hpad[:, PAD:PAD + L],
                                            xc[hc], rstd_sb, AL.mult)
                    # taps 1,5 on GpSimd (ts+ts+tt; fused stt is not
                    # supported on Pool by the compiler); taps 3,6 are
                    # scale-copies on ACT, combined on DVE in emit_achain.
                    accB = cnnd.tile([P, L], MM_DT, tag=f"dwB{hc}",
                                     name=f"dwB{i}_{hc}")
                    tB = cnnd.tile([P, L], MM_DT, tag=f"dwB2{hc}",
                                   name=f"dwB2{i}_{hc}")
                    nc.gpsimd.tensor_scalar(accB, hpad[:, 1:1 + L],
                                            dww_sb[:, i, hc, 1:2], None, AL.mult)
                    nc.gpsimd.tensor_scalar(tB, hpad[:, 5:5 + L],
                                            dww_sb[:, i, hc, 5:6], None, AL.mult)
                    nc.gpsimd.tensor_tensor(accB, accB, tB, AL.add)
                    tC3 = cnnd.tile([P, L], MM_DT, tag=f"dwC3{hc}",
                                    name=f"dwC3{i}_{hc}")
                    nc.scalar.activation(tC3, hpad[:, 3:3 + L], AF.Copy,
                                         scale=dww_sb[:, i, hc, 3:4])
                    tC6 = cnnd.tile([P, L], MM_DT, tag=f"dwC6{hc}",
                                    name=f"dwC6{i}_{hc}")
                    nc.scalar.activation(tC6, hpad[:, 6:6 + L], AF.Copy,
                                         scale=dww_sb[:, i, hc, 6:7])
                    return hpad, (accB, tC3, tC6)

                def emit_achain(hc, hpad, parts):
                    accB, tC3, tC6 = parts
                    accA = cnnd.tile([P, L], MM_DT, tag=f"dwA{hc}",
                                     name=f"dwA{i}_{hc}")
                    nc.vector.tensor_scalar(accA, hpad[:, 0:L],
                                            dww_sb[:, i, hc, 0:1], None, AL.mult)
                    for k in (2, 4):
                        tmp = cnnd.tile([P, L], MM_DT, tag=f"dwT{hc}",
                                        name=f"dwT{i}_{hc}_{k}")
                        nc.vector.tensor_scalar(tmp, hpad[:, k:k + L],
                                                dww_sb[:, i, hc, k:k + 1],
                                                None, AL.mult)
                        nc.vector.tensor_tensor(accA, accA, tmp, AL.add)
                    nc.vector.tensor_tensor(accA, accA, tC3, AL.add)
                    nc.vector.tensor_tensor(accA, accA, tC6, AL.add)
                    nc.vector.tensor_tensor(accA, accA, accB, AL.add)
                    return accA

                hp = [emit_hpad(0), emit_hpad(1)]
                dw_tiles = []
                for hc in range(HT):
                    if hc + 2 < HT:
                        hp.append(emit_hpad(hc + 2))
                    dw_tiles.append(emit_achain(hc, *hp[hc]))
                for lc in range(LC):
                    sl = slice(lc * 512, (lc + 1) * 512)
                    pms = [ps_cnn.tile([P, 512], dt.float32, tag="pw", bufs=5,
                                       name=f"pwps{i}_{lc}_{oc}")
                           for oc in range(HT)]
                    # kc-outer: PE starts on dw chunk 0 while later chunks
                    # are still being computed on DVE/GpSimd.
                    for kc in range(HT):
                        for oc in range(HT):
                            nc.tensor.matmul(pms[oc],
                                             pwT_sb[kc][:, oc * P:(oc + 1) * P],
                                             dw_tiles[kc][:, sl],
                                             start=(kc == 0), stop=False)
                    for oc in range(HT):
                        nc.tensor.matmul(pms[oc], uconv_sb[:, i, oc * P:(oc + 1) * P],
                                         mshift[:, sl], start=False, stop=True)
                        rl = work.tile([P, 512], MM_DT, tag="relu")
                        nc.scalar.activation(rl, pms[oc], AF.Relu,
                                             bias=pwb_sb[:, i, oc:oc + 1])
                        nc.vector.tensor_tensor(xc[oc][:, sl], xc[oc][:, sl], rl,
                                                AL.add)

        if stop_stage == "cnn":
            write_out(xc)
            nc.compile()
            return nc

        # =================== attention ===================
        with tc.tile_pool(name="attnp", bufs=1) as attnp, \
             tc.tile_pool(name="expp", bufs=2) as expp, \
             tc.tile_pool(name="ps_attn", bufs=1, space="PSUM") as ps_attn:
            wqk_sb = []
            for hc in range(HT):
                t = attnp.tile([P, NHEAD * 2 * DK], MM_DT, tag=f"wqk{hc}")
                nc.sync.dma_start(out=t, in_=d_wqk[hc * P:(hc + 1) * P, :])
                wqk_sb.append(t)
            qkb_sb = singles.tile([P, NHEAD], dt.float32)
            nc.sync.dma_start(out=qkb_sb, in_=d_qkb[:, :])
            wv_sb = []
            for hc in range(HT):
                t = attnp.tile([P, H], MM_DT, tag=f"wv{hc}")
                nc.sync.dma_start(out=t, in_=d_wv[hc * P:(hc + 1) * P, :])
                wv_sb.append(t)
            vbias_sb = singles.tile([1, H], MM_DT)
            nc.sync.dma_start(out=vbias_sb, in_=d_vbias[:, :])
            projT_sb = []
            for hc in range(HT):
                t = attnp.tile([P, H], MM_DT, tag=f"projT{hc}")
                nc.sync.dma_start(out=t, in_=d_projT[hc * P:(hc + 1) * P, :])
                projT_sb.append(t)
            projb_sb = singles.tile([P, HT], dt.float32)
            nc.sync.dma_start(out=projb_sb, in_=d_projb[:, :])

            rstd_sb, m_sb = ln_stats(ps_attn, stat_bufs=1, need_mean=True)
            xs = []
            for hc in range(HT):
                t = attnp.tile([P, L], MM_DT, tag=f"xs{hc}", name=f"xs{hc}")
                nc.gpsimd.tensor_tensor(t, xc[hc], m_sb, AL.subtract)
                nc.gpsimd.tensor_tensor(t, t, rstd_sb, AL.mult)
                xs.append(t)

            # QK projections: per head psum [q:0-63 | k:64-127, 512], then
            # split into base-0 tiles (matmul operands share base partition);
            # beta-fold biases added during the PSUM eviction copies.
            qT, kT = [], []
            for h in range(NHEAD):
                qt = attnp.tile([DK, L], MM_DT, tag=f"qT{h}", name=f"qT{h}")
                kt = attnp.tile([DK, L], MM_DT, tag=f"kT{h}", name=f"kT{h}")
                for lc in range(LC):
                    sl = slice(lc * 512, (lc + 1) * 512)
                    pm = ps_attn.tile([P, 512], dt.float32, tag="a", bufs=2,
                                      name=f"qkps{h}_{lc}")
                    for kc in range(HT):
                        nc.tensor.matmul(pm,
                                         wqk_sb[kc][:, h * 2 * DK:(h + 1) * 2 * DK],
                                         xs[kc][:, sl],
                                         start=(kc == 0), stop=(kc == HT - 1))
                    nc.vector.tensor_scalar(qt[:, sl], pm[0:DK, :],
                                            qkb_sb[0:DK, h:h + 1], None, AL.add)
                    nc.scalar.activation(kt[:, sl], pm[DK:2 * DK, :], AF.Identity,
                                         bias=qkb_sb[DK:2 * DK, h:h + 1])
                qT.append(qt)
                kT.append(kt)

            # V per key-chunk: vh_plus [128, 8, 65] (col 64 = ones)
            vh_plus = []
            for mc in range(MC):
                msl = slice(mc * P, (mc + 1) * P)
                pm = ps_attn.tile([P, 512], dt.float32, tag="a", bufs=2,
                                  name=f"vps{mc}")
                for kc in range(HT):
                    nc.tensor.matmul(pm, xs[kc][:, msl], wv_sb[kc],
                                     start=(kc == 0), stop=False)
                nc.tensor.matmul(pm, ones_row_bf, vbias_sb[0:1, :],
                                 start=False, stop=True)
                vp = attnp.tile([P, NHEAD, DK + 1], MM_DT, tag=f"vhp{mc}")
                nc.vector.tensor_copy(
                    vp[:, :, 0:DK], pm.rearrange("p (h v) -> p h v", h=NHEAD))
                nc.gpsimd.memset(vp[:, :, DK:DK + 1], 1.0)
                vh_plus.append(vp)

            oT = [attnp.tile([P, L], MM_DT, tag=f"oT{hc}", name=f"oT{hc}")
                  for hc in range(HT)]

            def emit_scores(h):
                expT = []
                for mc in range(MC):
                    et = expp.tile([P, L], MM_DT, tag=f"expT{mc}",
                                   name=f"expT{h}_{mc}")
                    msl = slice(mc * P, (mc + 1) * P)
                    pm = ps_attn.tile([P, L], dt.float32, tag="sc", bufs=2,
                                      name=f"scps{h}_{mc}")
                    for lc in range(LC):
                        sl = slice(lc * 512, (lc + 1) * 512)
                        nc.tensor.matmul(pm[:, sl], kT[h][:, msl], qT[h][:, sl],
                                         start=True, stop=True)
                    nc.scalar.activation(et, pm, AF.Exp,
                                         bias=maskb[:, mc:mc + 1],
                                         scale=1.0 / float(np.sqrt(DK)))
                    expT.append(et)
                return expT

            def emit_attnv(h, expT):
                rrow_bf = stats.tile([1, L], MM_DT, tag="rrowb",
                                     name=f"rrowb{h}")
                ohc, r0 = h // 2, (h % 2) * DK
                for lc in range(LC):
                    sl = slice(lc * 512, (lc + 1) * 512)
                    po = ps_attn.tile([DK + 1, 512], dt.float32, tag="a", bufs=2,
                                      name=f"po{h}_{lc}")
                    for mc in range(MC):
                        nc.tensor.matmul(po, vh_plus[mc][:, h, :], expT[mc][:, sl],
                                         start=(mc == 0), stop=(mc == MC - 1))
                    with nc.allow_low_precision(reason="softmax denom fits bf16"):
                        nc.vector.reciprocal(rrow_bf[:, sl], po[DK:DK + 1, :])
                    rbc = work.tile([DK, 512], MM_DT, tag="rbc")
                    nc.gpsimd.partition_broadcast(rbc, rrow_bf[0:1, sl])
                    orow = work.tile([DK, 512], MM_DT, tag="oraw")
                    nc.vector.tensor_copy(orow, po[0:DK, :])
                    nc.vector.tensor_tensor(oT[ohc][r0:r0 + DK, sl], orow, rbc,
                                            AL.mult)

            # software-pipelined head loop: scores/exp of head h issue ahead
            # of attnV of head h-1 so the PE never waits on the exp of the
            # current head.
            prev = None
            for h in range(NHEAD):
                expT = emit_scores(h)
                if prev is not None:
                    emit_attnv(h - 1, prev)
                prev = expT
            emit_attnv(NHEAD - 1, prev)

            for lc in range(LC):
                for oc in range(HT):
                    sl = slice(lc * 512, (lc + 1) * 512)
                    pm = ps_attn.tile([P, 512], dt.float32, tag="a", bufs=2,
                                      name=f"prps{oc}_{lc}")
                    for jc in range(HT):
                        nc.tensor.matmul(pm, projT_sb[jc][:, oc * P:(oc + 1) * P],
                                         oT[jc][:, sl],
                                         start=(jc == 0), stop=(jc == HT - 1))
                    prc = work.tile([P, 512], MM_DT, tag="prc")
                    nc.vector.tensor_scalar(prc, pm, projb_sb[:, oc:oc + 1],
                                            None, AL.add)
                    nc.gpsimd.tensor_tensor(xc[oc][:, sl], xc[oc][:, sl], prc,
                                            AL.add)

        if stop_stage == "attn":
            write_out(xc)
            nc.compile()
            return nc

        # =================== FFN ===================
        with tc.tile_pool(name="ffnp", bufs=1) as ffnp, \
             tc.tile_pool(name="ps_ffn", bufs=1, space="PSUM") as ps_ffn:
            w1T_sb = []
            for hc in range(HT):
                t = ffnp.tile([P, F], MM_DT, tag=f"w1T{hc}")
                nc.sync.dma_start(out=t, in_=d_w1T[hc * P:(hc + 1) * P, :])
                w1T_sb.append(t)
            f1b_sb = singles.tile([P, FT], dt.float32)
            nc.sync.dma_start(out=f1b_sb, in_=d_f1b[:, :])
            w2T_sb = []
            for fc in range(FT):
                t = ffnp.tile([P, H], MM_DT, tag=f"w2T{fc}")
                nc.sync.dma_start(out=t, in_=d_w2T[fc * P:(fc + 1) * P, :])
                w2T_sb.append(t)
            b2_sb = singles.tile([P, HT], dt.float32)
            nc.sync.dma_start(out=b2_sb, in_=d_b2[:, :])

            rstd_sb, m_sb = ln_stats(ps_ffn, stat_bufs=1, need_mean=True)
            xs2 = []
            for hc in range(HT):
                t = ffnp.tile([P, L], MM_DT, tag=f"xs2{hc}", name=f"xs2{hc}")
                nc.vector.tensor_tensor(t, xc[hc], m_sb, AL.subtract)
                nc.vector.tensor_tensor(t, t, rstd_sb, AL.mult)
                xs2.append(t)

            # ffn1 and the lc0 half of ffn2 interleave per fc (4 open ffn2
            # chains consume h1[fc] as it is produced); lc1 half follows.
            h1 = []
            sl0 = slice(0, 512)
            f2pms = [ps_ffn.tile([P, 512], dt.float32, tag="f2", bufs=4,
                                 name=f"f2ps0_{oc}") for oc in range(HT)]
            for fc in range(FT):
                ht = ffnp.tile([P, L], MM_DT, tag=f"h1_{fc}", name=f"h1_{fc}")
                for lc in range(LC):
                    sl = slice(lc * 512, (lc + 1) * 512)
                    pm = ps_ffn.tile([P, 512], dt.float32, tag="f1ps", bufs=2,
                                     name=f"f1ps{fc}_{lc}")
                    for kc in range(HT):
                        nc.tensor.matmul(pm, w1T_sb[kc][:, fc * P:(fc + 1) * P],
                                         xs2[kc][:, sl],
                                         start=(kc == 0), stop=(kc == HT - 1))
                    nc.scalar.activation(ht[:, sl], pm, AF.Relu,
                                         bias=f1b_sb[:, fc:fc + 1])
                h1.append(ht)
                for oc in range(HT):
                    nc.tensor.matmul(f2pms[oc], w2T_sb[fc][:, oc * P:(oc + 1) * P],
                                     ht[:, sl0],
                                     start=(fc == 0), stop=(fc == FT - 1))
            for oc in range(HT):
                ot = work.tile([P, 512], dt.float32, tag="outf")
                nc.vector.scalar_tensor_tensor(ot, f2pms[oc], b2_sb[:, oc:oc + 1],
                                               xc[oc][:, 0:512], AL.add, AL.add)
                nc.sync.dma_start(out=d_out[oc * P:(oc + 1) * P, 0:512], in_=ot)

            sl1 = slice(512, 1024)
            for oc in range(HT):
                pm = ps_ffn.tile([P, 512], dt.float32, tag="f2", bufs=4,
                                 name=f"f2ps1_{oc}")
                for fc in range(FT):
                    nc.tensor.matmul(pm, w2T_sb[fc][:, oc * P:(oc + 1) * P],
                                     h1[fc][:, sl1],
                                     start=(fc == 0), stop=(fc == FT - 1))
                ot = work.tile([P, 512], dt.float32, tag="outf")
                nc.vector.scalar_tensor_tensor(ot, pm, b2_sb[:, oc:oc + 1],
                                               xc[oc][:, sl1], AL.add, AL.add)
                nc.sync.dma_start(out=d_out[oc * P:(oc + 1) * P, sl1], in_=ot)
    nc.compile()
    return nc


def _prep_inputs(x, x_mask, pos_emb, cnn_gamma, cnn_beta, cnn_dw_w, cnn_dw_b,
                 cnn_pw_w, cnn_pw_b, attn_gamma, attn_beta, w_qs, w_ks, w_vs,
                 proj_w, proj_b, ffn_gamma, ffn_beta, ffn_w1, ffn_b1, ffn_w2, ffn_b2):
    """Host-side layout/dtype staging -> per-core input maps."""
    f32 = np.float32
    bf = BF16
    x = np.asarray(x, f32)
    pos = np.asarray(pos_emb, f32)[0, :L, :]          # [L, H]
    mask = np.asarray(x_mask, bool)

    def cols(v):  # [N] -> [128, N/128] partition-major columns
        v = np.asarray(v, f32)
        return np.ascontiguousarray(v.reshape(len(v) // P, P).T)

    sh = {}
    sh["posT"] = np.ascontiguousarray(pos.T).astype(bf)
    # CNN
    sh["pwT"] = np.stack([(np.asarray(cnn_pw_w[i], f32)
                           * np.asarray(cnn_gamma[i], f32)[None, :]).T
                          for i in range(NLAYERS)]).astype(bf)
    dww = np.stack([np.asarray(cnn_dw_w[i], f32).reshape(HT, P, KSZ).transpose(1, 0, 2)
                    for i in range(NLAYERS)], axis=1)        # [P, NL, HT, KSZ]
    sh["dww"] = np.ascontiguousarray(dww)
    # uconv[0:7,i,o]  = sum_c PW[o,c]*gamma[c]*w[c,k]   (mean-row conv)
    # uconv[7:14,i,o] = sum_c PW[o,c]*beta[c]*w[c,k]    (edge-exact beta conv)
    uc = np.zeros((2 * KSZ, NLAYERS, H), f32)
    pwb_eff = np.zeros((P, NLAYERS, HT), f32)
    for i in range(NLAYERS):
        pw = np.asarray(cnn_pw_w[i], f32)          # [o, c]
        w = np.asarray(cnn_dw_w[i], f32)           # [c, k]
        gi = np.asarray(cnn_gamma[i], f32)
        bi = np.asarray(cnn_beta[i], f32)
        uc[0:KSZ, i] = np.einsum("oc,c,ck->ko", pw, gi, w)
        uc[KSZ:2 * KSZ, i] = np.einsum("oc,c,ck->ko", pw, bi, w)
        pwb_eff[:, i, :] = cols(np.asarray(cnn_pw_b[i], f32)
                                + pw @ np.asarray(cnn_dw_b[i], f32))
    sh["uconv"] = uc.astype(bf)
    sh["pwb"] = np.ascontiguousarray(pwb_eff)
    # attention (gamma folded into weights; beta becomes additive biases)
    g = np.asarray(attn_gamma, f32)
    be = np.asarray(attn_beta, f32)
    wq = np.asarray(w_qs, f32)   # [8, H, DK]
    wk = np.asarray(w_ks, f32)
    wv = np.asarray(w_vs, f32)
    wqk = np.concatenate([wq * g[None, :, None], wk * g[None, :, None]], axis=2)
    sh["wqk"] = np.ascontiguousarray(
        wqk.transpose(1, 0, 2).reshape(H, NHEAD * 2 * DK)).astype(bf)
    qkb = np.concatenate([np.einsum("hdk,d->hk", wq, be),
                          np.einsum("hdk,d->hk", wk, be)], axis=1)  # [8, 128]
    sh["qkb"] = np.ascontiguousarray(qkb.T)                          # [128, 8] f32
    sh["wv"] = np.ascontiguousarray(
        (wv * g[None, :, None]).transpose(1, 0, 2).reshape(H, H)).astype(bf)
    sh["vbias"] = np.einsum("hdv,d->hv", wv, be).reshape(1, H).astype(bf)
    sh["projT"] = np.ascontiguousarray(np.asarray(proj_w, f32).T).astype(bf)
    sh["projb"] = cols(proj_b)
    # ffn
    fg = np.asarray(ffn_gamma, f32)
    fb = np.asarray(ffn_beta, f32)
    w1 = np.asarray(ffn_w1, f32)   # [F, H]
    w2 = np.asarray(ffn_w2, f32)   # [H, F]
    sh["w1T"] = np.ascontiguousarray((w1 * fg[None, :]).T).astype(bf)
    sh["f1b"] = cols(w1 @ fb + np.asarray(ffn_b1, f32))
    sh["w2T"] = np.ascontiguousarray(w2.T).astype(bf)
    sh["b2"] = cols(ffn_b2)

    in_maps = []
    for b in range(B):
        m = dict(sh)
        m["xT"] = np.ascontiguousarray(x[b].T).astype(bf)
        m["maskb"] = np.ascontiguousarray(
            np.where(mask[b], np.float32(NEG), np.float32(0.0)).reshape(MC, P).T)
        in_maps.append(m)
    return in_maps


def kernel(x, x_mask, pos_emb, cnn_gamma, cnn_beta, cnn_dw_w, cnn_dw_b,
           cnn_pw_w, cnn_pw_b, attn_gamma, attn_beta, w_qs, w_ks, w_vs,
           proj_w, proj_b, ffn_gamma, ffn_beta, ffn_w1, ffn_b1, ffn_w2, ffn_b2):
    from concourse.bass_utils import run_bass_kernel_spmd
    if "nc" not in _CACHE:
        _CACHE["nc"] = _nc_build("full")
    nc = _CACHE["nc"]
    in_maps = _prep_inputs(x, x_mask, pos_emb, cnn_gamma, cnn_beta, cnn_dw_w,
                           cnn_dw_b, cnn_pw_w, cnn_pw_b, attn_gamma, attn_beta,
                           w_qs, w_ks, w_vs, proj_w, proj_b, ffn_gamma, ffn_beta,
                           ffn_w1, ffn_b1, ffn_w2, ffn_b2)
    res = run_bass_kernel_spmd(nc, in_maps, core_ids=list(range(B)))
    out = np.stack([np.asarray(res.results[b]["out"], np.float32).T
                    for b in range(B)])
    return out


# revision 41
# speedup vs baseline: 1.0007x; 1.0007x over previous
"""nn_GBEncoderBlock on 8 Trainium2 NeuronCores.

Sharding: data-parallel over batch (B=8 -> 1 element/core, no collectives).

On-chip layout is channel-major xT [H=512, L=1024] (4 tiles of 128
partitions). Layernorms reduce over H (= partitions) via ones-vector
matmuls on the PE; mean/rstd rows are broadcast back to [128, L] with K=1
matmuls and cached as bf16 SBUF tiles so every elementwise consumer runs
in the DVE 2x (16-bit packed) mode. Gamma is folded into the following
matmul weights host-side; beta/bias terms become per-partition biases of
the PSUM-evicting activation ops.

Matmuls run in bf16 (fp32 PSUM accumulate). The depthwise conv runs as 7
scaled-shift taps spread across DVE (4x tensor_scalar + 2x tensor_tensor
pairs), GpSimd and ACT (scale-copies), with the LN shift term folded
through depthwise+pointwise into one K=14 matmul over shifted copies of
the mean row. Softmax: scores are computed transposed ([key, query]) so
the key mask is a per-partition bias of the fused exp activation; the
denominator comes free as a 65th row of the attn@V matmul via an appended
ones column in the stationary operand. The head loop is software
pipelined (scores/exp of head h issue ahead of attnV of head h-1).
"""

import contextlib
import numpy as np
import ml_dtypes

B, L, H = 8, 1024, 512
NHEAD, DK = 8, 64
KSZ, NLAYERS = 7, 4
F = 4 * H
EPS = 1e-6
P = 128
HT = H // P      # 4  h-chunks
LC = L // 512    # 2  l-chunks of 512
MC = L // P      # 8  key chunks of 128
FT = F // P      # 16 ffn chunks
NEG = -10000.0
LN_N = float(H)
PAD = KSZ // 2   # 3
LPAD = L + KSZ - 1
BF16 = ml_dtypes.bfloat16

_CACHE = {}


def _nc_build(stop_stage="full"):
    import concourse.bass as bass  # noqa: F401
    from concourse import bacc, mybir
    import concourse.tile as tile

    dt = mybir.dt
    MM_DT = dt.bfloat16
    AL = mybir.AluOpType
    AF = mybir.ActivationFunctionType

    nc = bacc.Bacc(None, target_bir_lowering=False, debug=False)

    # ---------------- DRAM I/O ----------------
    d_xT = nc.dram_tensor("xT", [H, L], MM_DT, kind="ExternalInput")
    d_posT = nc.dram_tensor("posT", [H, L], MM_DT, kind="ExternalInput")
    d_maskb = nc.dram_tensor("maskb", [P, MC], dt.float32, kind="ExternalInput")
    d_pwT = nc.dram_tensor("pwT", [NLAYERS, H, H], MM_DT, kind="ExternalInput")
    d_uconv = nc.dram_tensor("uconv", [2 * KSZ, NLAYERS, H], MM_DT, kind="ExternalInput")
    d_dww = nc.dram_tensor("dww", [P, NLAYERS, HT, KSZ], dt.float32, kind="ExternalInput")
    d_pwb = nc.dram_tensor("pwb", [P, NLAYERS, HT], dt.float32, kind="ExternalInput")
    d_wqk = nc.dram_tensor("wqk", [H, NHEAD * 2 * DK], MM_DT, kind="ExternalInput")
    d_qkb = nc.dram_tensor("qkb", [P, NHEAD], dt.float32, kind="ExternalInput")
    d_wv = nc.dram_tensor("wv", [H, H], MM_DT, kind="ExternalInput")
    d_vbias = nc.dram_tensor("vbias", [1, H], MM_DT, kind="ExternalInput")
    d_projT = nc.dram_tensor("projT", [H, H], MM_DT, kind="ExternalInput")
    d_projb = nc.dram_tensor("projb", [P, HT], dt.float32, kind="ExternalInput")
    d_w1T = nc.dram_tensor("w1T", [H, F], MM_DT, kind="ExternalInput")
    d_f1b = nc.dram_tensor("f1b", [P, FT], dt.float32, kind="ExternalInput")
    d_w2T = nc.dram_tensor("w2T", [F, H], MM_DT, kind="ExternalInput")
    d_b2 = nc.dram_tensor("b2", [P, HT], dt.float32, kind="ExternalInput")
    d_out = nc.dram_tensor("out", [H, L], dt.float32, kind="ExternalOutput")

    with tile.TileContext(nc) as tc, contextlib.ExitStack() as ctx:
        singles = ctx.enter_context(tc.tile_pool(name="singles", bufs=1))
        stats = ctx.enter_context(tc.tile_pool(name="stats", bufs=2))
        work = ctx.enter_context(tc.tile_pool(name="work", bufs=3))
        resid = ctx.enter_context(tc.tile_pool(name="resid", bufs=1))
        sqp = ctx.enter_context(tc.tile_pool(name="sqp", bufs=1))
        ps_bc = ctx.enter_context(tc.tile_pool(name="ps_bc", bufs=1, space="PSUM"))

        # ---- constants ----
        ones_col = singles.tile([P, 1], MM_DT)            # K=128 -> M=1 column sums
        nc.vector.memset(ones_col, 1.0)
        ones_row_f32 = singles.tile([1, P], dt.float32)    # K=1 -> M=128 broadcast
        nc.vector.memset(ones_row_f32, 1.0)
        ones_row_bf = singles.tile([1, P], MM_DT)
        nc.vector.memset(ones_row_bf, 1.0)
        ones_row_dk = singles.tile([1, DK], MM_DT)         # K=1 -> M=64 broadcast
        nc.vector.memset(ones_row_dk, 1.0)
        maskb = singles.tile([P, MC], dt.float32)
        nc.sync.dma_start(out=maskb, in_=d_maskb[:, :])

        # ---- residual stream xc = xT + posT ----
        xc = []
        for hc in range(HT):
            xt = work.tile([P, L], MM_DT, tag="ld_x")
            pt = work.tile([P, L], MM_DT, tag="ld_pos")
            nc.sync.dma_start(out=xt, in_=d_xT[hc * P:(hc + 1) * P, :])
            nc.sync.dma_start(out=pt, in_=d_posT[hc * P:(hc + 1) * P, :])
            xr = resid.tile([P, L], MM_DT, tag=f"xc{hc}")
            nc.vector.tensor_tensor(xr, xt, pt, AL.add)
            xc.append(xr)

        def ln_stats(ps, stat_bufs=2, need_mean=False, mm_out=None):
            """LN over channels.

            Returns (rstd_sb [128,L] bf16, m_sb [128,L] bf16 | None).
            If mm_out is given (a [1, LPAD] padded row), writes -mean*rstd
            into its [PAD:PAD+L] span.
            """
            sq = []
            for hc in range(HT):
                s = sqp.tile([P, L], MM_DT, tag=f"sq{hc}")
                nc.gpsimd.tensor_tensor(s, xc[hc], xc[hc], AL.mult)
                sq.append(s)
            rstd_f32 = stats.tile([1, L], dt.float32, tag="rstdf")
            rstd_sb = stats.tile([P, L], MM_DT, tag="rstd_sb", name="rstd_sb")
            m_sb = None
            if need_mean:
                m_sb = stats.tile([P, L], MM_DT, tag="m_sb", name="m_sb")
            for lc in range(LC):
                sl = slice(lc * 512, (lc + 1) * 512)
                p_sum = ps.tile([1, 512], dt.float32, tag="pstat",
                                bufs=stat_bufs, name=f"psum_{lc}")
                p_sq = ps.tile([1, 512], dt.float32, tag="pstat",
                               bufs=stat_bufs, name=f"psq_{lc}")
                for hc in range(HT):
                    nc.tensor.matmul(p_sum, ones_col, xc[hc][:, sl],
                                     start=(hc == 0), stop=(hc == HT - 1))
                for hc in range(HT):
                    nc.tensor.matmul(p_sq, ones_col, sq[hc][:, sl],
                                     start=(hc == 0), stop=(hc == HT - 1))
                sum_sb = stats.tile([1, 512], dt.float32, tag="sum_sb")
                nc.scalar.copy(sum_sb, p_sum)
                s2 = stats.tile([1, 512], dt.float32, tag="s2")
                nc.vector.tensor_tensor(s2, sum_sb, sum_sb, AL.mult)
                v1 = stats.tile([1, 512], dt.float32, tag="v1")
                nc.vector.tensor_scalar(v1, p_sq, 1.0 / (LN_N - 1.0), None, AL.mult)
                var = stats.tile([1, 512], dt.float32, tag="var")
                nc.vector.scalar_tensor_tensor(var, s2, -1.0 / (LN_N * (LN_N - 1.0)),
                                               v1, AL.mult, AL.add)
                std = stats.tile([1, 512], dt.float32, tag="std")
                nc.scalar.activation(std, var, AF.Sqrt)
                # eps dropped: std ~ O(1) here, 1e-6 is far below bf16 noise
                nc.vector.reciprocal(rstd_f32[:, sl], std)
                pb = ps_bc.tile([P, 512], dt.float32, tag="bc")
                nc.tensor.matmul(pb, ones_row_f32, rstd_f32[:, sl],
                                 start=True, stop=True)
                nc.scalar.copy(rstd_sb[:, sl], pb)
                if need_mean:
                    mrow = stats.tile([1, 512], dt.float32, tag="mrow")
                    nc.vector.tensor_scalar(mrow, sum_sb, 1.0 / LN_N, None, AL.mult)
                    pm2 = ps_bc.tile([P, 512], dt.float32, tag="bc")
                    nc.tensor.matmul(pm2, ones_row_f32, mrow, start=True, stop=True)
                    nc.scalar.copy(m_sb[:, sl], pm2)
                if mm_out is not None:
                    nc.vector.scalar_tensor_tensor(
                        mm_out[0:1, PAD + lc * 512:PAD + (lc + 1) * 512], sum_sb,
                        -1.0 / LN_N, rstd_f32[:, sl], AL.mult, AL.mult)
            return rstd_sb, m_sb

        def write_out(src_tiles):
            for hc in range(HT):
                ot = work.tile([P, L], dt.float32, tag="outt")
                nc.vector.tensor_copy(ot, src_tiles[hc])
                nc.sync.dma_start(out=d_out[hc * P:(hc + 1) * P, :], in_=ot)

        # =================== CNN layers ===================
        # LN shift term (-mean*rstd, beta) is folded through depthwise +
        # pointwise into a K=14 matmul over shifted copies of the mean row
        # (exact, incl. conv edge truncation); depthwise bias is folded into
        # the pointwise bias host-side. So hpad holds only (xc*gamma)*rstd.
        with tc.tile_pool(name="cnnw", bufs=2) as cnnw, \
             tc.tile_pool(name="cnnd", bufs=1) as cnnd, \
             tc.tile_pool(name="cnnm", bufs=2) as cnnm, \
             tc.tile_pool(name="ps_cnn", bufs=1, space="PSUM") as ps_cnn:
            uconv_sb = singles.tile([2 * KSZ, NLAYERS, H], MM_DT)
            nc.sync.dma_start(out=uconv_sb, in_=d_uconv[:, :, :])
            dww_sb = singles.tile([P, NLAYERS, HT, KSZ], dt.float32)
            nc.sync.dma_start(out=dww_sb, in_=d_dww[:, :, :, :])
            pwb_sb = singles.tile([P, NLAYERS, HT], dt.float32)
            nc.sync.dma_start(out=pwb_sb, in_=d_pwb[:, :, :])
            ones_pad = singles.tile([1, LPAD], MM_DT)
            nc.vector.memset(ones_pad[0:1, :], 0.0)
            nc.vector.memset(ones_pad[0:1, PAD:PAD + L], 1.0)

            for i in range(NLAYERS):
                pwT_sb = []
                for hc in range(HT):
                    t = cnnw.tile([P, H], MM_DT, tag=f"pwT{hc}")
                    nc.sync.dma_start(out=t, in_=d_pwT[i, hc * P:(hc + 1) * P, :])
                    pwT_sb.append(t)
                mm_pad = cnnm.tile([1, LPAD], MM_DT, tag="mm_pad",
                                   name=f"mm_pad{i}")
                nc.vector.memset(mm_pad[0:1, 0:PAD], 0.0)
                nc.vector.memset(mm_pad[0:1, LPAD - PAD:LPAD], 0.0)
                rstd_sb, _ = ln_stats(ps_cnn, mm_out=mm_pad)
                # shifted-window gather [14, L]: rows 0-6 = mm shifts,
                # rows 7-13 = ones-pad shifts (edge-exact beta term)
                mshift = cnnm.tile([2 * KSZ, L], MM_DT, tag="mshift",
                                   name=f"mshift{i}")
                mm_win = bass.AP(tensor=mm_pad.tensor, offset=mm_pad.offset,
                                 ap=[[1, 1], [1, KSZ], [1, L]])
                nc.sync.dma_start(out=mshift[0:KSZ, :], in_=mm_win)
                ones_win = bass.AP(tensor=ones_pad.tensor, offset=ones_pad.offset,
                                   ap=[[1, 1], [1, KSZ], [1, L]])
                nc.sync.dma_start(out=mshift[KSZ:2 * KSZ, :], in_=ones_win)
                # per-hc pipeline: hpad_hc -> GpSimd B-chain while DVE runs
                # the A-chain; combine right after so dw_hc frees early for
                # the kc-outer pointwise matmuls.
                def emit_hpad(hc):
                    hpad = cnnd.tile([P, LPAD], MM_DT, tag=f"hpad{hc}",
                                     name=f"hpad{i}_{hc}")
                    nc.vector.memset(hpad[:, 0:PAD], 0.0)
                    nc.vector.memset(hpad[:, LPAD - PAD:LPAD], 0.0)
                    nc.vector.tensor_tensor(hpad[:, PAD:PAD + L],
                                            xc[hc], rstd_sb, AL.mult)
                    # taps 1,5 on GpSimd (ts+ts+tt; fused stt is not
                    # supported on Pool by the compiler); taps 3,6 are
                    # scale-copies on ACT, combined on DVE in emit_achain.
                    accB = cnnd.tile([P, L], MM_DT, tag=f"dwB{hc}",
                                     name=f"dwB{i}_{hc}")
                    tB = cnnd.tile([P, L], MM_DT, tag=f"dwB2{hc}",
                                   name=f"dwB2{i}_{hc}")
                    nc.gpsimd.tensor_scalar(accB, hpad[:, 1:1 + L],
                                            dww_sb[:, i, hc, 1:2], None, AL.mult)
                    nc.gpsimd.tensor_scalar(tB, hpad[:, 5:5 + L],
                                            dww_sb[:, i, hc, 5:6], None, AL.mult)
                    nc.gpsimd.tensor_tensor(accB, accB, tB, AL.add)
                    tC3 = cnnd.tile([P, L], MM_DT, tag=f"dwC3{hc}",
                                    name=f"dwC3{i}_{hc}")
                    nc.scalar.activation(tC3, hpad[:, 3:3 + L], AF.Copy,
                                         scale=dww_sb[:, i, hc, 3:4])
                    tC6 = cnnd.tile([P, L], MM_DT, tag=f"dwC6{hc}",
                                    name=f"dwC6{i}_{hc}")
                    nc.scalar.activation(tC6, hpad[:, 6:6 + L], AF.Copy,
                                         scale=dww_sb[:, i, hc, 6:7])
                    return hpad, (accB, tC3, tC6)

                def emit_achain(hc, hpad, parts):
                    accB, tC3, tC6 = parts
                    accA = cnnd.tile([P, L], MM_DT, tag=f"dwA{hc}",
                                     name=f"dwA{i}_{hc}")
                    nc.vector.tensor_scalar(accA, hpad[:, 0:L],
                                            dww_sb[:, i, hc, 0:1], None, AL.mult)
                    for k in (2, 4):
                        tmp = cnnd.tile([P, L], MM_DT, tag=f"dwT{hc}",
                                        name=f"dwT{i}_{hc}_{k}")
                        nc.vector.tensor_scalar(tmp, hpad[:, k:k + L],
                                                dww_sb[:, i, hc, k:k + 1],
                                                None, AL.mult)
                        nc.vector.tensor_tensor(accA, accA, tmp, AL.add)
                    nc.vector.tensor_tensor(accA, accA, tC3, AL.add)
                    nc.vector.tensor_tensor(accA, accA, tC6, AL.add)
                    nc.vector.tensor_tensor(accA, accA, accB, AL.add)
                    return accA

                hp = [emit_hpad(0), emit_hpad(1)]
                dw_tiles = []
                for hc in range(HT):
                    if hc + 2 < HT:
                        hp.append(emit_hpad(hc + 2))
                    dw_tiles.append(emit_achain(hc, *hp[hc]))
                for lc in range(LC):
                    sl = slice(lc * 512, (lc + 1) * 512)
                    pms = [ps_cnn.tile([P, 512], dt.float32, tag="pw", bufs=5,
                                       name=f"pwps{i}_{lc}_{oc}")
                           for oc in range(HT)]
                    # kc-outer: PE starts on dw chunk 0 while later chunks
                    # are still being computed on DVE/GpSimd.
                    for kc in range(HT):
                        for oc in range(HT):
                            nc.tensor.matmul(pms[oc],
                                             pwT_sb[kc][:, oc * P:(oc + 1) * P],
                                             dw_tiles[kc][:, sl],
                                             start=(kc == 0), stop=False)
                    for oc in range(HT):
                        nc.tensor.matmul(pms[oc], uconv_sb[:, i, oc * P:(oc + 1) * P],
                                         mshift[:, sl], start=False, stop=True)
                        rl = work.tile([P, 512], MM_DT, tag="relu")
                        nc.scalar.activation(rl, pms[oc], AF.Relu,
                                             bias=pwb_sb[:, i, oc:oc + 1])
                        nc.gpsimd.tensor_tensor(xc[oc][:, sl], xc[oc][:, sl], rl,
                                                AL.add)

        if stop_stage == "cnn":
            write_out(xc)
            nc.compile()
            return nc

        # =================== attention ===================
        with tc.tile_pool(name="attnp", bufs=1) as attnp, \
             tc.tile_pool(name="expp", bufs=2) as expp, \
             tc.tile_pool(name="ps_attn", bufs=1, space="PSUM") as ps_attn:
            wqk_sb = []
            for hc in range(HT):
                t = attnp.tile([P, NHEAD * 2 * DK], MM_DT, tag=f"wqk{hc}")
                nc.sync.dma_start(out=t, in_=d_wqk[hc * P:(hc + 1) * P, :])
                wqk_sb.append(t)
            qkb_sb = singles.tile([P, NHEAD], dt.float32)
            nc.sync.dma_start(out=qkb_sb, in_=d_qkb[:, :])
            wv_sb = []
            for hc in range(HT):
                t = attnp.tile([P, H], MM_DT, tag=f"wv{hc}")
                nc.sync.dma_start(out=t, in_=d_wv[hc * P:(hc + 1) * P, :])
                wv_sb.append(t)
            vbias_sb = singles.tile([1, H], MM_DT)
            nc.sync.dma_start(out=vbias_sb, in_=d_vbias[:, :])
            projT_sb = []
            for hc in range(HT):
                t = attnp.tile([P, H], MM_DT, tag=f"projT{hc}")
                nc.sync.dma_start(out=t, in_=d_projT[hc * P:(hc + 1) * P, :])
                projT_sb.append(t)
            projb_sb = singles.tile([P, HT], dt.float32)
            nc.sync.dma_start(out=projb_sb, in_=d_projb[:, :])

            rstd_sb, m_sb = ln_stats(ps_attn, stat_bufs=1, need_mean=True)
            xs = []
            for hc in range(HT):
                t = attnp.tile([P, L], MM_DT, tag=f"xs{hc}", name=f"xs{hc}")
                nc.gpsimd.tensor_tensor(t, xc[hc], m_sb, AL.subtract)
                nc.gpsimd.tensor_tensor(t, t, rstd_sb, AL.mult)
                xs.append(t)

            # QK projections: per head psum [q:0-63 | k:64-127, 512], then
            # split into base-0 tiles (matmul operands share base partition);
            # beta-fold biases added during the PSUM eviction copies.
            qT, kT = [], []
            for h in range(NHEAD):
                qt = attnp.tile([DK, L], MM_DT, tag=f"qT{h}", name=f"qT{h}")
                kt = attnp.tile([DK, L], MM_DT, tag=f"kT{h}", name=f"kT{h}")
                for lc in range(LC):
                    sl = slice(lc * 512, (lc + 1) * 512)
                    pm = ps_attn.tile([P, 512], dt.float32, tag="a", bufs=2,
                                      name=f"qkps{h}_{lc}")
                    for kc in range(HT):
                        nc.tensor.matmul(pm,
                                         wqk_sb[kc][:, h * 2 * DK:(h + 1) * 2 * DK],
                                         xs[kc][:, sl],
                                         start=(kc == 0), stop=(kc == HT - 1))
                    nc.vector.tensor_scalar(qt[:, sl], pm[0:DK, :],
                                            qkb_sb[0:DK, h:h + 1], None, AL.add)
                    nc.vector.tensor_scalar(kt[:, sl], pm[DK:2 * DK, :],
                                            qkb_sb[DK:2 * DK, h:h + 1], None, AL.add)
                qT.append(qt)
                kT.append(kt)

            # V per key-chunk: vh_plus [128, 8, 65] (col 64 = ones)
            vh_plus = []
            for mc in range(MC):
                msl = slice(mc * P, (mc + 1) * P)
                pm = ps_attn.tile([P, 512], dt.float32, tag="a", bufs=2,
                                  name=f"vps{mc}")
                for kc in range(HT):
                    nc.tensor.matmul(pm, xs[kc][:, msl], wv_sb[kc],
                                     start=(kc == 0), stop=False)
                nc.tensor.matmul(pm, ones_row_bf, vbias_sb[0:1, :],
                                 start=False, stop=True)
                vp = attnp.tile([P, NHEAD, DK + 1], MM_DT, tag=f"vhp{mc}")
                nc.vector.tensor_copy(
                    vp[:, :, 0:DK], pm.rearrange("p (h v) -> p h v", h=NHEAD))
                nc.gpsimd.memset(vp[:, :, DK:DK + 1], 1.0)
                vh_plus.append(vp)

            oT = [attnp.tile([P, L], MM_DT, tag=f"oT{hc}", name=f"oT{hc}")
                  for hc in range(HT)]

            def emit_scores(h):
                expT = []
                for mc in range(MC):
                    et = expp.tile([P, L], MM_DT, tag=f"expT{mc}",
                                   name=f"expT{h}_{mc}")
                    msl = slice(mc * P, (mc + 1) * P)
                    pm = ps_attn.tile([P, L], dt.float32, tag="sc", bufs=2,
                                      name=f"scps{h}_{mc}")
                    for lc in range(LC):
                        sl = slice(lc * 512, (lc + 1) * 512)
                        nc.tensor.matmul(pm[:, sl], kT[h][:, msl], qT[h][:, sl],
                                         start=True, stop=True)
                    nc.scalar.activation(et, pm, AF.Exp,
                                         bias=maskb[:, mc:mc + 1],
                                         scale=1.0 / float(np.sqrt(DK)))
                    expT.append(et)
                return expT

            def emit_attnv(h, expT):
                rrow_bf = stats.tile([1, L], MM_DT, tag="rrowb",
                                     name=f"rrowb{h}")
                ohc, r0 = h // 2, (h % 2) * DK
                for lc in range(LC):
                    sl = slice(lc * 512, (lc + 1) * 512)
                    po = ps_attn.tile([DK + 1, 512], dt.float32, tag="a", bufs=2,
                                      name=f"po{h}_{lc}")
                    for mc in range(MC):
                        nc.tensor.matmul(po, vh_plus[mc][:, h, :], expT[mc][:, sl],
                                         start=(mc == 0), stop=(mc == MC - 1))
                    with nc.allow_low_precision(reason="softmax denom fits bf16"):
                        nc.vector.reciprocal(rrow_bf[:, sl], po[DK:DK + 1, :])
                    rbc = work.tile([DK, 512], MM_DT, tag="rbc")
                    nc.gpsimd.partition_broadcast(rbc, rrow_bf[0:1, sl])
                    orow = work.tile([DK, 512], MM_DT, tag="oraw")
                    nc.vector.tensor_copy(orow, po[0:DK, :])
                    nc.vector.tensor_tensor(oT[ohc][r0:r0 + DK, sl], orow, rbc,
                                            AL.mult)

            # software-pipelined head loop: scores/exp of head h issue ahead
            # of attnV of head h-1 so the PE never waits on the exp of the
            # current head.
            prev = None
            for h in range(NHEAD):
                expT = emit_scores(h)
                if prev is not None:
                    emit_attnv(h - 1, prev)
                prev = expT
            emit_attnv(NHEAD - 1, prev)

            for lc in range(LC):
                for oc in range(HT):
                    sl = slice(lc * 512, (lc + 1) * 512)
                    pm = ps_attn.tile([P, 512], dt.float32, tag="a", bufs=2,
                                      name=f"prps{oc}_{lc}")
                    for jc in range(HT):
                        nc.tensor.matmul(pm, projT_sb[jc][:, oc * P:(oc + 1) * P],
                                         oT[jc][:, sl],
                                         start=(jc == 0), stop=(jc == HT - 1))
                    prc = work.tile([P, 512], MM_DT, tag="prc")
                    nc.vector.tensor_scalar(prc, pm, projb_sb[:, oc:oc + 1],
                                            None, AL.add)
                    nc.gpsimd.tensor_tensor(xc[oc][:, sl], xc[oc][:, sl], prc,
                                            AL.add)

        if stop_stage == "attn":
            write_out(xc)
            nc.compile()
            return nc

        # =================== FFN ===================
        with tc.tile_pool(name="ffnp", bufs=1) as ffnp, \
             tc.tile_pool(name="ps_ffn", bufs=1, space="PSUM") as ps_ffn:
            w1T_sb = []
            for hc in range(HT):
                t = ffnp.tile([P, F], MM_DT, tag=f"w1T{hc}")
                nc.sync.dma_start(out=t, in_=d_w1T[hc * P:(hc + 1) * P, :])
                w1T_sb.append(t)
            f1b_sb = singles.tile([P, FT], dt.float32)
            nc.sync.dma_start(out=f1b_sb, in_=d_f1b[:, :])
            w2T_sb = []
            for fc in range(FT):
                t = ffnp.tile([P, H], MM_DT, tag=f"w2T{fc}")
                nc.sync.dma_start(out=t, in_=d_w2T[fc * P:(fc + 1) * P, :])
                w2T_sb.append(t)
            b2_sb = singles.tile([P, HT], dt.float32)
            nc.sync.dma_start(out=b2_sb, in_=d_b2[:, :])

            rstd_sb, m_sb = ln_stats(ps_ffn, stat_bufs=1, need_mean=True)
            xs2 = []
            for hc in range(HT):
                t = ffnp.tile([P, L], MM_DT, tag=f"xs2{hc}", name=f"xs2{hc}")
                nc.vector.tensor_tensor(t, xc[hc], m_sb, AL.subtract)
                nc.vector.tensor_tensor(t, t, rstd_sb, AL.mult)
                xs2.append(t)

            # ffn1 and the lc0 half of ffn2 interleave per fc (4 open ffn2
            # chains consume h1[fc] as it is produced); lc1 half follows.
            h1 = []
            sl0 = slice(0, 512)
            f2pms = [ps_ffn.tile([P, 512], dt.float32, tag="f2", bufs=4,
                                 name=f"f2ps0_{oc}") for oc in range(HT)]
            for fc in range(FT):
                ht = ffnp.tile([P, L], MM_DT, tag=f"h1_{fc}", name=f"h1_{fc}")
                for lc in range(LC):
                    sl = slice(lc * 512, (lc + 1) * 512)
                    pm = ps_ffn.tile([P, 512], dt.float32, tag="f1ps", bufs=2,
                                     name=f"f1ps{fc}_{lc}")
                    for kc in range(HT):
                        nc.tensor.matmul(pm, w1T_sb[kc][:, fc * P:(fc + 1) * P],
                                         xs2[kc][:, sl],
                                         start=(kc == 0), stop=(kc == HT - 1))
                    nc.scalar.activation(ht[:, sl], pm, AF.Relu,
                                         bias=f1b_sb[:, fc:fc + 1])
                h1.append(ht)
                for oc in range(HT):
                    nc.tensor.matmul(f2pms[oc], w2T_sb[fc][:, oc * P:(oc + 1) * P],
                                     ht[:, sl0],
                                     start=(fc == 0), stop=(fc == FT - 1))
            for oc in range(HT):
                ot = work.tile([P, 512], dt.float32, tag="outf")
                nc.vector.scalar_tensor_tensor(ot, f2pms[oc], b2_sb[:, oc:oc + 1],
                                               xc[oc][:, 0:512], AL.add, AL.add)
                nc.sync.dma_start(out=d_out[oc * P:(oc + 1) * P, 0:512], in_=ot)

            sl1 = slice(512, 1024)
            for oc in range(HT):
                pm = ps_ffn.tile([P, 512], dt.float32, tag="f2", bufs=4,
                                 name=f"f2ps1_{oc}")
                for fc in range(FT):
                    nc.tensor.matmul(pm, w2T_sb[fc][:, oc * P:(oc + 1) * P],
                                     h1[fc][:, sl1],
                                     start=(fc == 0), stop=(fc == FT - 1))
                ot = work.tile([P, 512], dt.float32, tag="outf")
                nc.vector.scalar_tensor_tensor(ot, pm, b2_sb[:, oc:oc + 1],
                                               xc[oc][:, sl1], AL.add, AL.add)
                nc.sync.dma_start(out=d_out[oc * P:(oc + 1) * P, sl1], in_=ot)
    nc.compile()
    return nc


def _prep_inputs(x, x_mask, pos_emb, cnn_gamma, cnn_beta, cnn_dw_w, cnn_dw_b,
                 cnn_pw_w, cnn_pw_b, attn_gamma, attn_beta, w_qs, w_ks, w_vs,
                 proj_w, proj_b, ffn_gamma, ffn_beta, ffn_w1, ffn_b1, ffn_w2, ffn_b2):
    """Host-side layout/dtype staging -> per-core input maps."""
    f32 = np.float32
    bf = BF16
    x = np.asarray(x, f32)
    pos = np.asarray(pos_emb, f32)[0, :L, :]          # [L, H]
    mask = np.asarray(x_mask, bool)

    def cols(v):  # [N] -> [128, N/128] partition-major columns
        v = np.asarray(v, f32)
        return np.ascontiguousarray(v.reshape(len(v) // P, P).T)

    sh = {}
    sh["posT"] = np.ascontiguousarray(pos.T).astype(bf)
    # CNN
    sh["pwT"] = np.stack([(np.asarray(cnn_pw_w[i], f32)
                           * np.asarray(cnn_gamma[i], f32)[None, :]).T
                          for i in range(NLAYERS)]).astype(bf)
    dww = np.stack([np.asarray(cnn_dw_w[i], f32).reshape(HT, P, KSZ).transpose(1, 0, 2)
                    for i in range(NLAYERS)], axis=1)        # [P, NL, HT, KSZ]
    sh["dww"] = np.ascontiguousarray(dww)
    # uconv[0:7,i,o]  = sum_c PW[o,c]*gamma[c]*w[c,k]   (mean-row conv)
    # uconv[7:14,i,o] = sum_c PW[o,c]*beta[c]*w[c,k]    (edge-exact beta conv)
    uc = np.zeros((2 * KSZ, NLAYERS, H), f32)
    pwb_eff = np.zeros((P, NLAYERS, HT), f32)
    for i in range(NLAYERS):
        pw = np.asarray(cnn_pw_w[i], f32)          # [o, c]
        w = np.asarray(cnn_dw_w[i], f32)           # [c, k]
        gi = np.asarray(cnn_gamma[i], f32)
        bi = np.asarray(cnn_beta[i], f32)
        uc[0:KSZ, i] = np.einsum("oc,c,ck->ko", pw, gi, w)
        uc[KSZ:2 * KSZ, i] = np.einsum("oc,c,ck->ko", pw, bi, w)
        pwb_eff[:, i, :] = cols(np.asarray(cnn_pw_b[i], f32)
                                + pw @ np.asarray(cnn_dw_b[i], f32))
    sh["uconv"] = uc.astype(bf)
    sh["pwb"] = np.ascontiguousarray(pwb_eff)
    # attention (gamma folded into weights; beta becomes additive biases)
    g = np.asarray(attn_gamma, f32)
    be = np.asarray(attn_beta, f32)
    wq = np.asarray(w_qs, f32)   # [8, H, DK]
    wk = np.asarray(w_ks, f32)
    wv = np.asarray(w_vs, f32)
    wqk = np.concatenate([wq * g[None, :, None], wk * g[None, :, None]], axis=2)
    sh["wqk"] = np.ascontiguousarray(
        wqk.transpose(1, 0, 2).reshape(H, NHEAD * 2 * DK)).astype(bf)
    qkb = np.concatenate([np.einsum("hdk,d->hk", wq, be),
                          np.einsum("hdk,d->hk", wk, be)], axis=1)  # [8, 128]
    sh["qkb"] = np.ascontiguousarray(qkb.T)                          # [128, 8] f32
    sh["wv"] = np.ascontiguousarray(
        (wv * g[None, :, None]).transpose(1, 0, 2).reshape(H, H)).astype(bf)
    sh["vbias"] = np.einsum("hdv,d->hv", wv, be).reshape(1, H).astype(bf)
    sh["projT"] = np.ascontiguousarray(np.asarray(proj_w, f32).T).astype(bf)
    sh["projb"] = cols(proj_b)
    # ffn
    fg = np.asarray(ffn_gamma, f32)
    fb = np.asarray(ffn_beta, f32)
    w1 = np.asarray(ffn_w1, f32)   # [F, H]
    w2 = np.asarray(ffn_w2, f32)   # [H, F]
    sh["w1T"] = np.ascontiguousarray((w1 * fg[None, :]).T).astype(bf)
    sh["f1b"] = cols(w1 @ fb + np.asarray(ffn_b1, f32))
    sh["w2T"] = np.ascontiguousarray(w2.T).astype(bf)
    sh["b2"] = cols(ffn_b2)

    in_maps = []
    for b in range(B):
        m = dict(sh)
        m["xT"] = np.ascontiguousarray(x[b].T).astype(bf)
        m["maskb"] = np.ascontiguousarray(
            np.where(mask[b], np.float32(NEG), np.float32(0.0)).reshape(MC, P).T)
        in_maps.append(m)
    return in_maps


def kernel(x, x_mask, pos_emb, cnn_gamma, cnn_beta, cnn_dw_w, cnn_dw_b,
           cnn_pw_w, cnn_pw_b, attn_gamma, attn_beta, w_qs, w_ks, w_vs,
           proj_w, proj_b, ffn_gamma, ffn_beta, ffn_w1, ffn_b1, ffn_w2, ffn_b2):
    from concourse.bass_utils import run_bass_kernel_spmd
    if "nc" not in _CACHE:
        _CACHE["nc"] = _nc_build("full")
    nc = _CACHE["nc"]
    in_maps = _prep_inputs(x, x_mask, pos_emb, cnn_gamma, cnn_beta, cnn_dw_w,
                           cnn_dw_b, cnn_pw_w, cnn_pw_b, attn_gamma, attn_beta,
                           w_qs, w_ks, w_vs, proj_w, proj_b, ffn_gamma, ffn_beta,
                           ffn_w1, ffn_b1, ffn_w2, ffn_b2)
    res = run_bass_kernel_spmd(nc, in_maps, core_ids=list(range(B)))
    out = np.stack([np.asarray(res.results[b]["out"], np.float32).T
                    for b in range(B)])
    return out


# revision 43
# speedup vs baseline: 1.0167x; 1.0159x over previous
"""nn_GBEncoderBlock on 8 Trainium2 NeuronCores.

Sharding: data-parallel over batch (B=8 -> 1 element/core, no collectives).

On-chip layout is channel-major xT [H=512, L=1024] (4 tiles of 128
partitions). Layernorms reduce over H (= partitions) via ones-vector
matmuls on the PE; mean/rstd rows are broadcast back to [128, L] with K=1
matmuls and cached as bf16 SBUF tiles so every elementwise consumer runs
in the DVE 2x (16-bit packed) mode. Gamma is folded into the following
matmul weights host-side; beta/bias terms become per-partition biases of
the PSUM-evicting activation ops.

Matmuls run in bf16 (fp32 PSUM accumulate). The depthwise conv runs as 7
scaled-shift taps spread across DVE (4x tensor_scalar + 2x tensor_tensor
pairs), GpSimd and ACT (scale-copies), with the LN shift term folded
through depthwise+pointwise into one K=14 matmul over shifted copies of
the mean row. Softmax: scores are computed transposed ([key, query]) so
the key mask is a per-partition bias of the fused exp activation; the
denominator comes free as a 65th row of the attn@V matmul via an appended
ones column in the stationary operand. The head loop is software
pipelined (scores/exp of head h issue ahead of attnV of head h-1).
"""

import contextlib
import numpy as np
import ml_dtypes

B, L, H = 8, 1024, 512
NHEAD, DK = 8, 64
KSZ, NLAYERS = 7, 4
F = 4 * H
EPS = 1e-6
P = 128
HT = H // P      # 4  h-chunks
LC = L // 512    # 2  l-chunks of 512
MC = L // P      # 8  key chunks of 128
FT = F // P      # 16 ffn chunks
NEG = -10000.0
LN_N = float(H)
PAD = KSZ // 2   # 3
LPAD = L + KSZ - 1
BF16 = ml_dtypes.bfloat16

_CACHE = {}


def _nc_build(stop_stage="full"):
    import concourse.bass as bass  # noqa: F401
    from concourse import bacc, mybir
    import concourse.tile as tile

    dt = mybir.dt
    MM_DT = dt.bfloat16
    AL = mybir.AluOpType
    AF = mybir.ActivationFunctionType

    nc = bacc.Bacc(None, target_bir_lowering=False, debug=False)

    # ---------------- DRAM I/O ----------------
    d_xT = nc.dram_tensor("xT", [H, L], MM_DT, kind="ExternalInput")
    d_posT = nc.dram_tensor("posT", [H, L], MM_DT, kind="ExternalInput")
    d_maskb = nc.dram_tensor("maskb", [P, MC], dt.float32, kind="ExternalInput")
    d_pwT = nc.dram_tensor("pwT", [NLAYERS, H, H], MM_DT, kind="ExternalInput")
    d_uconv = nc.dram_tensor("uconv", [2 * KSZ, NLAYERS, H], MM_DT, kind="ExternalInput")
    d_dww = nc.dram_tensor("dww", [P, NLAYERS, HT, KSZ], dt.float32, kind="ExternalInput")
    d_pwb = nc.dram_tensor("pwb", [P, NLAYERS, HT], dt.float32, kind="ExternalInput")
    d_wqk = nc.dram_tensor("wqk", [H, NHEAD * 2 * DK], MM_DT, kind="ExternalInput")
    d_qkb = nc.dram_tensor("qkb", [P, NHEAD], dt.float32, kind="ExternalInput")
    d_wv = nc.dram_tensor("wv", [H, H], MM_DT, kind="ExternalInput")
    d_vbias = nc.dram_tensor("vbias", [1, H], MM_DT, kind="ExternalInput")
    d_projT = nc.dram_tensor("projT", [H, H], MM_DT, kind="ExternalInput")
    d_projb = nc.dram_tensor("projb", [P, HT], dt.float32, kind="ExternalInput")
    d_w1T = nc.dram_tensor("w1T", [H, F], MM_DT, kind="ExternalInput")
    d_f1b = nc.dram_tensor("f1b", [P, FT], dt.float32, kind="ExternalInput")
    d_w2T = nc.dram_tensor("w2T", [F, H], MM_DT, kind="ExternalInput")
    d_b2 = nc.dram_tensor("b2", [P, HT], dt.float32, kind="ExternalInput")
    d_out = nc.dram_tensor("out", [H, L], dt.float32, kind="ExternalOutput")

    with tile.TileContext(nc) as tc, contextlib.ExitStack() as ctx:
        singles = ctx.enter_context(tc.tile_pool(name="singles", bufs=1))
        stats = ctx.enter_context(tc.tile_pool(name="stats", bufs=2))
        work = ctx.enter_context(tc.tile_pool(name="work", bufs=3))
        resid = ctx.enter_context(tc.tile_pool(name="resid", bufs=1))
        sqp = ctx.enter_context(tc.tile_pool(name="sqp", bufs=1))
        ps_bc = ctx.enter_context(tc.tile_pool(name="ps_bc", bufs=1, space="PSUM"))

        # ---- constants ----
        ones_col = singles.tile([P, 1], MM_DT)            # K=128 -> M=1 column sums
        nc.vector.memset(ones_col, 1.0)
        ones_row_f32 = singles.tile([1, P], dt.float32)    # K=1 -> M=128 broadcast
        nc.vector.memset(ones_row_f32, 1.0)
        ones_row_bf = singles.tile([1, P], MM_DT)
        nc.vector.memset(ones_row_bf, 1.0)
        ones_row_dk = singles.tile([1, DK], MM_DT)         # K=1 -> M=64 broadcast
        nc.vector.memset(ones_row_dk, 1.0)
        maskb = singles.tile([P, MC], dt.float32)
        nc.sync.dma_start(out=maskb, in_=d_maskb[:, :])

        # ---- residual stream xc = xT + posT ----
        xc = []
        for hc in range(HT):
            xt = work.tile([P, L], MM_DT, tag="ld_x")
            pt = work.tile([P, L], MM_DT, tag="ld_pos")
            nc.sync.dma_start(out=xt, in_=d_xT[hc * P:(hc + 1) * P, :])
            nc.sync.dma_start(out=pt, in_=d_posT[hc * P:(hc + 1) * P, :])
            xr = resid.tile([P, L], MM_DT, tag=f"xc{hc}")
            nc.vector.tensor_tensor(xr, xt, pt, AL.add)
            xc.append(xr)

        def ln_stats(ps, stat_bufs=2, need_mean=False, mm_out=None):
            """LN over channels.

            Returns (rstd_sb [128,L] bf16, m_sb [128,L] bf16 | None).
            If mm_out is given (a [1, LPAD] padded row), writes -mean*rstd
            into its [PAD:PAD+L] span.
            """
            sq = []
            for hc in range(HT):
                s = sqp.tile([P, L], MM_DT, tag=f"sq{hc}")
                for lc in range(LC):
                    sl = slice(lc * 512, (lc + 1) * 512)
                    nc.gpsimd.tensor_tensor(s[:, sl], xc[hc][:, sl],
                                            xc[hc][:, sl], AL.mult)
                sq.append(s)
            rstd_f32 = stats.tile([1, L], dt.float32, tag="rstdf")
            rstd_sb = stats.tile([P, L], MM_DT, tag="rstd_sb", name="rstd_sb")
            m_sb = None
            if need_mean:
                m_sb = stats.tile([P, L], MM_DT, tag="m_sb", name="m_sb")
            for lc in range(LC):
                sl = slice(lc * 512, (lc + 1) * 512)
                p_sum = ps.tile([1, 512], dt.float32, tag="pstat",
                                bufs=stat_bufs, name=f"psum_{lc}")
                p_sq = ps.tile([1, 512], dt.float32, tag="pstat",
                               bufs=stat_bufs, name=f"psq_{lc}")
                for hc in range(HT):
                    nc.tensor.matmul(p_sum, ones_col, xc[hc][:, sl],
                                     start=(hc == 0), stop=(hc == HT - 1))
                for hc in range(HT):
                    nc.tensor.matmul(p_sq, ones_col, sq[hc][:, sl],
                                     start=(hc == 0), stop=(hc == HT - 1))
                sum_sb = stats.tile([1, 512], dt.float32, tag="sum_sb")
                nc.scalar.copy(sum_sb, p_sum)
                s2 = stats.tile([1, 512], dt.float32, tag="s2")
                nc.vector.tensor_tensor(s2, sum_sb, sum_sb, AL.mult)
                v1 = stats.tile([1, 512], dt.float32, tag="v1")
                nc.vector.tensor_scalar(v1, p_sq, 1.0 / (LN_N - 1.0), None, AL.mult)
                var = stats.tile([1, 512], dt.float32, tag="var")
                nc.vector.scalar_tensor_tensor(var, s2, -1.0 / (LN_N * (LN_N - 1.0)),
                                               v1, AL.mult, AL.add)
                std = stats.tile([1, 512], dt.float32, tag="std")
                nc.scalar.activation(std, var, AF.Sqrt)
                # eps dropped: std ~ O(1) here, 1e-6 is far below bf16 noise
                nc.vector.reciprocal(rstd_f32[:, sl], std)
                pb = ps_bc.tile([P, 512], dt.float32, tag="bc")
                nc.tensor.matmul(pb, ones_row_f32, rstd_f32[:, sl],
                                 start=True, stop=True)
                nc.scalar.copy(rstd_sb[:, sl], pb)
                if need_mean:
                    mrow = stats.tile([1, 512], dt.float32, tag="mrow")
                    nc.vector.tensor_scalar(mrow, sum_sb, 1.0 / LN_N, None, AL.mult)
                    pm2 = ps_bc.tile([P, 512], dt.float32, tag="bc")
                    nc.tensor.matmul(pm2, ones_row_f32, mrow, start=True, stop=True)
                    nc.scalar.copy(m_sb[:, sl], pm2)
                if mm_out is not None:
                    nc.vector.scalar_tensor_tensor(
                        mm_out[0:1, PAD + lc * 512:PAD + (lc + 1) * 512], sum_sb,
                        -1.0 / LN_N, rstd_f32[:, sl], AL.mult, AL.mult)
            return rstd_sb, m_sb

        def write_out(src_tiles):
            for hc in range(HT):
                ot = work.tile([P, L], dt.float32, tag="outt")
                nc.vector.tensor_copy(ot, src_tiles[hc])
                nc.sync.dma_start(out=d_out[hc * P:(hc + 1) * P, :], in_=ot)

        # =================== CNN layers ===================
        # LN shift term (-mean*rstd, beta) is folded through depthwise +
        # pointwise into a K=14 matmul over shifted copies of the mean row
        # (exact, incl. conv edge truncation); depthwise bias is folded into
        # the pointwise bias host-side. So hpad holds only (xc*gamma)*rstd.
        with tc.tile_pool(name="cnnw", bufs=2) as cnnw, \
             tc.tile_pool(name="cnnd", bufs=1) as cnnd, \
             tc.tile_pool(name="cnnm", bufs=2) as cnnm, \
             tc.tile_pool(name="ps_cnn", bufs=1, space="PSUM") as ps_cnn:
            uconv_sb = singles.tile([2 * KSZ, NLAYERS, H], MM_DT)
            nc.sync.dma_start(out=uconv_sb, in_=d_uconv[:, :, :])
            dww_sb = singles.tile([P, NLAYERS, HT, KSZ], dt.float32)
            nc.sync.dma_start(out=dww_sb, in_=d_dww[:, :, :, :])
            pwb_sb = singles.tile([P, NLAYERS, HT], dt.float32)
            nc.sync.dma_start(out=pwb_sb, in_=d_pwb[:, :, :])
            ones_pad = singles.tile([1, LPAD], MM_DT)
            nc.vector.memset(ones_pad[0:1, :], 0.0)
            nc.vector.memset(ones_pad[0:1, PAD:PAD + L], 1.0)

            for i in range(NLAYERS):
                pwT_sb = []
                for hc in range(HT):
                    t = cnnw.tile([P, H], MM_DT, tag=f"pwT{hc}")
                    nc.sync.dma_start(out=t, in_=d_pwT[i, hc * P:(hc + 1) * P, :])
                    pwT_sb.append(t)
                mm_pad = cnnm.tile([1, LPAD], MM_DT, tag="mm_pad",
                                   name=f"mm_pad{i}")
                nc.vector.memset(mm_pad[0:1, 0:PAD], 0.0)
                nc.vector.memset(mm_pad[0:1, LPAD - PAD:LPAD], 0.0)
                rstd_sb, _ = ln_stats(ps_cnn, mm_out=mm_pad)
                # shifted-window gather [14, L]: rows 0-6 = mm shifts,
                # rows 7-13 = ones-pad shifts (edge-exact beta term)
                mshift = cnnm.tile([2 * KSZ, L], MM_DT, tag="mshift",
                                   name=f"mshift{i}")
                mm_win = bass.AP(tensor=mm_pad.tensor, offset=mm_pad.offset,
                                 ap=[[1, 1], [1, KSZ], [1, L]])
                nc.sync.dma_start(out=mshift[0:KSZ, :], in_=mm_win)
                ones_win = bass.AP(tensor=ones_pad.tensor, offset=ones_pad.offset,
                                   ap=[[1, 1], [1, KSZ], [1, L]])
                nc.sync.dma_start(out=mshift[KSZ:2 * KSZ, :], in_=ones_win)
                # per-hc pipeline: hpad_hc -> GpSimd B-chain while DVE runs
                # the A-chain; combine right after so dw_hc frees early for
                # the kc-outer pointwise matmuls.
                def emit_hpad(hc):
                    hpad = cnnd.tile([P, LPAD], MM_DT, tag=f"hpad{hc}",
                                     name=f"hpad{i}_{hc}")
                    nc.vector.memset(hpad[:, 0:PAD], 0.0)
                    nc.vector.memset(hpad[:, LPAD - PAD:LPAD], 0.0)
                    nc.vector.tensor_tensor(hpad[:, PAD:PAD + L],
                                            xc[hc], rstd_sb, AL.mult)
                    # taps 1,5 on GpSimd (ts+ts+tt; fused stt is not
                    # supported on Pool by the compiler); taps 3,6 are
                    # scale-copies on ACT, combined on DVE in emit_achain.
                    accB = cnnd.tile([P, L], MM_DT, tag=f"dwB{hc}",
                                     name=f"dwB{i}_{hc}")
                    tB = cnnd.tile([P, L], MM_DT, tag=f"dwB2{hc}",
                                   name=f"dwB2{i}_{hc}")
                    nc.gpsimd.tensor_scalar(accB, hpad[:, 1:1 + L],
                                            dww_sb[:, i, hc, 1:2], None, AL.mult)
                    nc.gpsimd.tensor_scalar(tB, hpad[:, 5:5 + L],
                                            dww_sb[:, i, hc, 5:6], None, AL.mult)
                    nc.gpsimd.tensor_tensor(accB, accB, tB, AL.add)
                    tC3 = cnnd.tile([P, L], MM_DT, tag=f"dwC3{hc}",
                                    name=f"dwC3{i}_{hc}")
                    nc.scalar.activation(tC3, hpad[:, 3:3 + L], AF.Copy,
                                         scale=dww_sb[:, i, hc, 3:4])
                    tC6 = cnnd.tile([P, L], MM_DT, tag=f"dwC6{hc}",
                                    name=f"dwC6{i}_{hc}")
                    nc.scalar.activation(tC6, hpad[:, 6:6 + L], AF.Copy,
                                         scale=dww_sb[:, i, hc, 6:7])
                    return hpad, (accB, tC3, tC6)

                def emit_achain(hc, hpad, parts):
                    accB, tC3, tC6 = parts
                    accA = cnnd.tile([P, L], MM_DT, tag=f"dwA{hc}",
                                     name=f"dwA{i}_{hc}")
                    nc.vector.tensor_scalar(accA, hpad[:, 0:L],
                                            dww_sb[:, i, hc, 0:1], None, AL.mult)
                    for k in (2, 4):
                        tmp = cnnd.tile([P, L], MM_DT, tag=f"dwT{hc}",
                                        name=f"dwT{i}_{hc}_{k}")
                        nc.vector.tensor_scalar(tmp, hpad[:, k:k + L],
                                                dww_sb[:, i, hc, k:k + 1],
                                                None, AL.mult)
                        nc.vector.tensor_tensor(accA, accA, tmp, AL.add)
                    nc.vector.tensor_tensor(accA, accA, tC3, AL.add)
                    nc.vector.tensor_tensor(accA, accA, tC6, AL.add)
                    nc.vector.tensor_tensor(accA, accA, accB, AL.add)
                    return accA

                hp = [emit_hpad(0), emit_hpad(1)]
                dw_tiles = []
                for hc in range(HT):
                    if hc + 2 < HT:
                        hp.append(emit_hpad(hc + 2))
                    dw_tiles.append(emit_achain(hc, *hp[hc]))
                for lc in range(LC):
                    sl = slice(lc * 512, (lc + 1) * 512)
                    pms = [ps_cnn.tile([P, 512], dt.float32, tag="pw", bufs=5,
                                       name=f"pwps{i}_{lc}_{oc}")
                           for oc in range(HT)]
                    # kc-outer: PE starts on dw chunk 0 while later chunks
                    # are still being computed on DVE/GpSimd.
                    for kc in range(HT):
                        for oc in range(HT):
                            nc.tensor.matmul(pms[oc],
                                             pwT_sb[kc][:, oc * P:(oc + 1) * P],
                                             dw_tiles[kc][:, sl],
                                             start=(kc == 0), stop=False)
                    for oc in range(HT):
                        nc.tensor.matmul(pms[oc], uconv_sb[:, i, oc * P:(oc + 1) * P],
                                         mshift[:, sl], start=False, stop=True)
                        rl = work.tile([P, 512], MM_DT, tag="relu")
                        nc.scalar.activation(rl, pms[oc], AF.Relu,
                                             bias=pwb_sb[:, i, oc:oc + 1])
                        nc.gpsimd.tensor_tensor(xc[oc][:, sl], xc[oc][:, sl], rl,
                                                AL.add)

        if stop_stage == "cnn":
            write_out(xc)
            nc.compile()
            return nc

        # =================== attention ===================
        with tc.tile_pool(name="attnp", bufs=1) as attnp, \
             tc.tile_pool(name="expp", bufs=2) as expp, \
             tc.tile_pool(name="ps_attn", bufs=1, space="PSUM") as ps_attn:
            wqk_sb = []
            for hc in range(HT):
                t = attnp.tile([P, NHEAD * 2 * DK], MM_DT, tag=f"wqk{hc}")
                nc.sync.dma_start(out=t, in_=d_wqk[hc * P:(hc + 1) * P, :])
                wqk_sb.append(t)
            qkb_sb = singles.tile([P, NHEAD], dt.float32)
            nc.sync.dma_start(out=qkb_sb, in_=d_qkb[:, :])
            wv_sb = []
            for hc in range(HT):
                t = attnp.tile([P, H], MM_DT, tag=f"wv{hc}")
                nc.sync.dma_start(out=t, in_=d_wv[hc * P:(hc + 1) * P, :])
                wv_sb.append(t)
            vbias_sb = singles.tile([1, H], MM_DT)
            nc.sync.dma_start(out=vbias_sb, in_=d_vbias[:, :])
            projT_sb = []
            for hc in range(HT):
                t = attnp.tile([P, H], MM_DT, tag=f"projT{hc}")
                nc.sync.dma_start(out=t, in_=d_projT[hc * P:(hc + 1) * P, :])
                projT_sb.append(t)
            projb_sb = singles.tile([P, HT], dt.float32)
            nc.sync.dma_start(out=projb_sb, in_=d_projb[:, :])

            rstd_sb, m_sb = ln_stats(ps_attn, stat_bufs=1, need_mean=True)
            xs = []
            for hc in range(HT):
                t = attnp.tile([P, L], MM_DT, tag=f"xs{hc}", name=f"xs{hc}")
                nc.gpsimd.tensor_tensor(t, xc[hc], m_sb, AL.subtract)
                nc.gpsimd.tensor_tensor(t, t, rstd_sb, AL.mult)
                xs.append(t)

            # QK projections: per head psum [q:0-63 | k:64-127, 512], then
            # split into base-0 tiles (matmul operands share base partition);
            # beta-fold biases added during the PSUM eviction copies.
            qT, kT = [], []
            for h in range(NHEAD):
                qt = attnp.tile([DK, L], MM_DT, tag=f"qT{h}", name=f"qT{h}")
                kt = attnp.tile([DK, L], MM_DT, tag=f"kT{h}", name=f"kT{h}")
                for lc in range(LC):
                    sl = slice(lc * 512, (lc + 1) * 512)
                    pm = ps_attn.tile([P, 512], dt.float32, tag="a", bufs=2,
                                      name=f"qkps{h}_{lc}")
                    for kc in range(HT):
                        nc.tensor.matmul(pm,
                                         wqk_sb[kc][:, h * 2 * DK:(h + 1) * 2 * DK],
                                         xs[kc][:, sl],
                                         start=(kc == 0), stop=(kc == HT - 1))
                    nc.vector.tensor_scalar(qt[:, sl], pm[0:DK, :],
                                            qkb_sb[0:DK, h:h + 1], None, AL.add)
                    nc.scalar.activation(kt[:, sl], pm[DK:2 * DK, :], AF.Identity,
                                         bias=qkb_sb[DK:2 * DK, h:h + 1])
                qT.append(qt)
                kT.append(kt)

            # V per key-chunk: vh_plus [128, 8, 65] (col 64 = ones)
            vh_plus = []
            for mc in range(MC):
                msl = slice(mc * P, (mc + 1) * P)
                pm = ps_attn.tile([P, 512], dt.float32, tag="a", bufs=2,
                                  name=f"vps{mc}")
                for kc in range(HT):
                    nc.tensor.matmul(pm, xs[kc][:, msl], wv_sb[kc],
                                     start=(kc == 0), stop=False)
                nc.tensor.matmul(pm, ones_row_bf, vbias_sb[0:1, :],
                                 start=False, stop=True)
                vp = attnp.tile([P, NHEAD, DK + 1], MM_DT, tag=f"vhp{mc}")
                nc.scalar.copy(
                    vp[:, :, 0:DK], pm.rearrange("p (h v) -> p h v", h=NHEAD))
                nc.gpsimd.memset(vp[:, :, DK:DK + 1], 1.0)
                vh_plus.append(vp)

            oT = [attnp.tile([P, L], MM_DT, tag=f"oT{hc}", name=f"oT{hc}")
                  for hc in range(HT)]

            def emit_scores(h):
                expT = []
                for mc in range(MC):
                    et = expp.tile([P, L], MM_DT, tag=f"expT{mc}",
                                   name=f"expT{h}_{mc}")
                    msl = slice(mc * P, (mc + 1) * P)
                    pm = ps_attn.tile([P, L], dt.float32, tag="sc", bufs=2,
                                      name=f"scps{h}_{mc}")
                    for lc in range(LC):
                        sl = slice(lc * 512, (lc + 1) * 512)
                        nc.tensor.matmul(pm[:, sl], kT[h][:, msl], qT[h][:, sl],
                                         start=True, stop=True)
                    nc.scalar.activation(et, pm, AF.Exp,
                                         bias=maskb[:, mc:mc + 1],
                                         scale=1.0 / float(np.sqrt(DK)))
                    expT.append(et)
                return expT

            def emit_attnv(h, expT):
                rrow_bf = stats.tile([1, L], MM_DT, tag="rrowb",
                                     name=f"rrowb{h}")
                ohc, r0 = h // 2, (h % 2) * DK
                for lc in range(LC):
                    sl = slice(lc * 512, (lc + 1) * 512)
                    po = ps_attn.tile([DK + 1, 512], dt.float32, tag="a", bufs=2,
                                      name=f"po{h}_{lc}")
                    for mc in range(MC):
                        nc.tensor.matmul(po, vh_plus[mc][:, h, :], expT[mc][:, sl],
                                         start=(mc == 0), stop=(mc == MC - 1))
                    with nc.allow_low_precision(reason="softmax denom fits bf16"):
                        nc.vector.reciprocal(rrow_bf[:, sl], po[DK:DK + 1, :])
                    rbc = work.tile([DK, 512], MM_DT, tag="rbc")
                    nc.gpsimd.partition_broadcast(rbc, rrow_bf[0:1, sl])
                    orow = work.tile([DK, 512], MM_DT, tag="oraw")
                    nc.vector.tensor_copy(orow, po[0:DK, :])
                    nc.vector.tensor_tensor(oT[ohc][r0:r0 + DK, sl], orow, rbc,
                                            AL.mult)

            # software-pipelined head loop: scores/exp of head h issue ahead
            # of attnV of head h-1 so the PE never waits on the exp of the
            # current head.
            prev = None
            for h in range(NHEAD):
                expT = emit_scores(h)
                if prev is not None:
                    emit_attnv(h - 1, prev)
                prev = expT
            emit_attnv(NHEAD - 1, prev)

            for lc in range(LC):
                for oc in range(HT):
                    sl = slice(lc * 512, (lc + 1) * 512)
                    pm = ps_attn.tile([P, 512], dt.float32, tag="a", bufs=2,
                                      name=f"prps{oc}_{lc}")
                    for jc in range(HT):
                        nc.tensor.matmul(pm, projT_sb[jc][:, oc * P:(oc + 1) * P],
                                         oT[jc][:, sl],
                                         start=(jc == 0), stop=(jc == HT - 1))
                    prc = work.tile([P, 512], MM_DT, tag="prc")
                    nc.vector.tensor_scalar(prc, pm, projb_sb[:, oc:oc + 1],
                                            None, AL.add)
                    nc.gpsimd.tensor_tensor(xc[oc][:, sl], xc[oc][:, sl], prc,
                                            AL.add)

        if stop_stage == "attn":
            write_out(xc)
            nc.compile()
            return nc

        # =================== FFN ===================
        with tc.tile_pool(name="ffnp", bufs=1) as ffnp, \
             tc.tile_pool(name="ps_ffn", bufs=1, space="PSUM") as ps_ffn:
            w1T_sb = []
            for hc in range(HT):
                t = ffnp.tile([P, F], MM_DT, tag=f"w1T{hc}")
                nc.sync.dma_start(out=t, in_=d_w1T[hc * P:(hc + 1) * P, :])
                w1T_sb.append(t)
            f1b_sb = singles.tile([P, FT], dt.float32)
            nc.sync.dma_start(out=f1b_sb, in_=d_f1b[:, :])
            w2T_sb = []
            for fc in range(FT):
                t = ffnp.tile([P, H], MM_DT, tag=f"w2T{fc}")
                nc.sync.dma_start(out=t, in_=d_w2T[fc * P:(fc + 1) * P, :])
                w2T_sb.append(t)
            b2_sb = singles.tile([P, HT], dt.float32)
            nc.sync.dma_start(out=b2_sb, in_=d_b2[:, :])

            rstd_sb, m_sb = ln_stats(ps_ffn, stat_bufs=1, need_mean=True)
            xs2 = []
            for hc in range(HT):
                t = ffnp.tile([P, L], MM_DT, tag=f"xs2{hc}", name=f"xs2{hc}")
                nc.vector.tensor_tensor(t, xc[hc], m_sb, AL.subtract)
                nc.vector.tensor_tensor(t, t, rstd_sb, AL.mult)
                xs2.append(t)

            # ffn1 and the lc0 half of ffn2 interleave per fc (4 open ffn2
            # chains consume h1[fc] as it is produced); lc1 half follows.
            h1 = []
            sl0 = slice(0, 512)
            f2pms = [ps_ffn.tile([P, 512], dt.float32, tag="f2", bufs=4,
                                 name=f"f2ps0_{oc}") for oc in range(HT)]
            for fc in range(FT):
                ht = ffnp.tile([P, L], MM_DT, tag=f"h1_{fc}", name=f"h1_{fc}")
                for lc in range(LC):
                    sl = slice(lc * 512, (lc + 1) * 512)
                    pm = ps_ffn.tile([P, 512], dt.float32, tag="f1ps", bufs=2,
                                     name=f"f1ps{fc}_{lc}")
                    for kc in range(HT):
                        nc.tensor.matmul(pm, w1T_sb[kc][:, fc * P:(fc + 1) * P],
                                         xs2[kc][:, sl],
                                         start=(kc == 0), stop=(kc == HT - 1))
                    nc.scalar.activation(ht[:, sl], pm, AF.Relu,
                                         bias=f1b_sb[:, fc:fc + 1])
                h1.append(ht)
                for oc in range(HT):
                    nc.tensor.matmul(f2pms[oc], w2T_sb[fc][:, oc * P:(oc + 1) * P],
                                     ht[:, sl0],
                                     start=(fc == 0), stop=(fc == FT - 1))
            for oc in range(HT):
                ot = work.tile([P, 512], dt.float32, tag="outf")
                nc.vector.scalar_tensor_tensor(ot, f2pms[oc], b2_sb[:, oc:oc + 1],
                                               xc[oc][:, 0:512], AL.add, AL.add)
                nc.sync.dma_start(out=d_out[oc * P:(oc + 1) * P, 0:512], in_=ot)

            sl1 = slice(512, 1024)
            for oc in range(HT):
                pm = ps_ffn.tile([P, 512], dt.float32, tag="f2", bufs=4,
                                 name=f"f2ps1_{oc}")
                for fc in range(FT):
                    nc.tensor.matmul(pm, w2T_sb[fc][:, oc * P:(oc + 1) * P],
                                     h1[fc][:, sl1],
                                     start=(fc == 0), stop=(fc == FT - 1))
                ot = work.tile([P, 512], dt.float32, tag="outf")
                nc.vector.scalar_tensor_tensor(ot, pm, b2_sb[:, oc:oc + 1],
                                               xc[oc][:, sl1], AL.add, AL.add)
                nc.sync.dma_start(out=d_out[oc * P:(oc + 1) * P, sl1], in_=ot)
    nc.compile()
    return nc


def _prep_inputs(x, x_mask, pos_emb, cnn_gamma, cnn_beta, cnn_dw_w, cnn_dw_b,
                 cnn_pw_w, cnn_pw_b, attn_gamma, attn_beta, w_qs, w_ks, w_vs,
                 proj_w, proj_b, ffn_gamma, ffn_beta, ffn_w1, ffn_b1, ffn_w2, ffn_b2):
    """Host-side layout/dtype staging -> per-core input maps."""
    f32 = np.float32
    bf = BF16
    x = np.asarray(x, f32)
    pos = np.asarray(pos_emb, f32)[0, :L, :]          # [L, H]
    mask = np.asarray(x_mask, bool)

    def cols(v):  # [N] -> [128, N/128] partition-major columns
        v = np.asarray(v, f32)
        return np.ascontiguousarray(v.reshape(len(v) // P, P).T)

    sh = {}
    sh["posT"] = np.ascontiguousarray(pos.T).astype(bf)
    # CNN
    sh["pwT"] = np.stack([(np.asarray(cnn_pw_w[i], f32)
                           * np.asarray(cnn_gamma[i], f32)[None, :]).T
                          for i in range(NLAYERS)]).astype(bf)
    dww = np.stack([np.asarray(cnn_dw_w[i], f32).reshape(HT, P, KSZ).transpose(1, 0, 2)
                    for i in range(NLAYERS)], axis=1)        # [P, NL, HT, KSZ]
    sh["dww"] = np.ascontiguousarray(dww)
    # uconv[0:7,i,o]  = sum_c PW[o,c]*gamma[c]*w[c,k]   (mean-row conv)
    # uconv[7:14,i,o] = sum_c PW[o,c]*beta[c]*w[c,k]    (edge-exact beta conv)
    uc = np.zeros((2 * KSZ, NLAYERS, H), f32)
    pwb_eff = np.zeros((P, NLAYERS, HT), f32)
    for i in range(NLAYERS):
        pw = np.asarray(cnn_pw_w[i], f32)          # [o, c]
        w = np.asarray(cnn_dw_w[i], f32)           # [c, k]
        gi = np.asarray(cnn_gamma[i], f32)
        bi = np.asarray(cnn_beta[i], f32)
        uc[0:KSZ, i] = np.einsum("oc,c,ck->ko", pw, gi, w)
        uc[KSZ:2 * KSZ, i] = np.einsum("oc,c,ck->ko", pw, bi, w)
        pwb_eff[:, i, :] = cols(np.asarray(cnn_pw_b[i], f32)
                                + pw @ np.asarray(cnn_dw_b[i], f32))
    sh["uconv"] = uc.astype(bf)
    sh["pwb"] = np.ascontiguousarray(pwb_eff)
    # attention (gamma folded into weights; beta becomes additive biases)
    g = np.asarray(attn_gamma, f32)
    be = np.asarray(attn_beta, f32)
    wq = np.asarray(w_qs, f32)   # [8, H, DK]
    wk = np.asarray(w_ks, f32)
    wv = np.asarray(w_vs, f32)
    wqk = np.concatenate([wq * g[None, :, None], wk * g[None, :, None]], axis=2)
    sh["wqk"] = np.ascontiguousarray(
        wqk.transpose(1, 0, 2).reshape(H, NHEAD * 2 * DK)).astype(bf)
    qkb = np.concatenate([np.einsum("hdk,d->hk", wq, be),
                          np.einsum("hdk,d->hk", wk, be)], axis=1)  # [8, 128]
    sh["qkb"] = np.ascontiguousarray(qkb.T)                          # [128, 8] f32
    sh["wv"] = np.ascontiguousarray(
        (wv * g[None, :, None]).transpose(1, 0, 2).reshape(H, H)).astype(bf)
    sh["vbias"] = np.einsum("hdv,d->hv", wv, be).reshape(1, H).astype(bf)
    sh["projT"] = np.ascontiguousarray(np.asarray(proj_w, f32).T).astype(bf)
    sh["projb"] = cols(proj_b)
    # ffn
    fg = np.asarray(ffn_gamma, f32)
    fb = np.asarray(ffn_beta, f32)
    w1 = np.asarray(ffn_w1, f32)   # [F, H]
    w2 = np.asarray(ffn_w2, f32)   # [H, F]
    sh["w1T"] = np.ascontiguousarray((w1 * fg[None, :]).T).astype(bf)
    sh["f1b"] = cols(w1 @ fb + np.asarray(ffn_b1, f32))
    sh["w2T"] = np.ascontiguousarray(w2.T).astype(bf)
    sh["b2"] = cols(ffn_b2)

    in_maps = []
    for b in range(B):
        m = dict(sh)
        m["xT"] = np.ascontiguousarray(x[b].T).astype(bf)
        m["maskb"] = np.ascontiguousarray(
            np.where(mask[b], np.float32(NEG), np.float32(0.0)).reshape(MC, P).T)
        in_maps.append(m)
    return in_maps


def kernel(x, x_mask, pos_emb, cnn_gamma, cnn_beta, cnn_dw_w, cnn_dw_b,
           cnn_pw_w, cnn_pw_b, attn_gamma, attn_beta, w_qs, w_ks, w_vs,
           proj_w, proj_b, ffn_gamma, ffn_beta, ffn_w1, ffn_b1, ffn_w2, ffn_b2):
    from concourse.bass_utils import run_bass_kernel_spmd
    if "nc" not in _CACHE:
        _CACHE["nc"] = _nc_build("full")
    nc = _CACHE["nc"]
    in_maps = _prep_inputs(x, x_mask, pos_emb, cnn_gamma, cnn_beta, cnn_dw_w,
                           cnn_dw_b, cnn_pw_w, cnn_pw_b, attn_gamma, attn_beta,
                           w_qs, w_ks, w_vs, proj_w, proj_b, ffn_gamma, ffn_beta,
                           ffn_w1, ffn_b1, ffn_w2, ffn_b2)
    res = run_bass_kernel_spmd(nc, in_maps, core_ids=list(range(B)))
    out = np.stack([np.asarray(res.results[b]["out"], np.float32).T
                    for b in range(B)])
    return out
